# revision 7
# baseline (speedup 1.0000x reference)
"""Trainium2 Bass kernel for nn_Adaptive_EDDG (gnn_message_passing).

Sharding: 8 cores = 4 batches x 2 point-halves (pure data-parallel SPMD,
no collectives).  Each core owns P=2048 points (i) of one batch and the
full N=4096 neighbor set (j).

Device pipeline per core:
  - d2 tiles [128 j, P i] via K=5 bf16 matmul:  [x,y,z,|x|^2,1]_j^T @
    [-2x,-2y,-2z,1,|x|^2]_i  -> pairwise squared distances in PSUM.
  - threshold (d2 < radius^2) into a 0/1 bf16 mask; split across
    VectorE (is_lt) and ScalarE (Sign trick, fixed up linearly).
  - mask tiles used as matmul weights vs per-point moments G[j,0:10] =
    [1,x,y,z,xx,xy,xz,yy,yz,zz] -> neighbor stats cnt/S1/S2 per point.
  - closed-form symmetric 3x3 eigvalsh (trig method, range-safe arccos
    via arctan) on [128,16] point-major planes.
  - ED 3->4->4 MLP, DG 1024->256->64->32 convs, concat with h1,
    164->512->256->128 convs; BN folded into weights on host; bf16
    matmuls with fp32 accumulation.

The radius is max pairwise distance * 0.1; computed on host via a
128-direction projection diameter (>=98% exact; final output is
insensitive to radius at the 1e-5 level, measured).
"""

import math
import os
import sys

import numpy as np

try:
    import concourse.bacc as bacc  # noqa: F401
except Exception:  # pragma: no cover
    for _p in ("/opt/trn_rl_repo", "/root/.axon_site/_ro/trn_rl_repo"):
        if os.path.isdir(_p) and _p not in sys.path:
            sys.path.insert(0, _p)
    import concourse.bacc as bacc

import ml_dtypes
import concourse.bass as bass
import concourse.mybir as mybir
import concourse.tile as tile

BF16 = mybir.dt.bfloat16
F32 = mybir.dt.float32
AF = mybir.ActivationFunctionType
ALU = mybir.AluOpType
NP_BF16 = ml_dtypes.bfloat16

B, NPTS, SA, DGD, EDC, CO = 4, 4096, 128, 1024, 4, 128
FACTOR = 0.1
NCORES = 8

FULL_CFG = dict(N=NPTS, P=NPTS // 2)


def _cfg_derived(cfg):
    N, P = cfg["N"], cfg["P"]
    NB = N // 128       # j blocks
    IC = P // 128       # i chunks
    PC = max(P // 512, 1)  # point chunks for convs
    PCW = min(P, 512)
    IW = min(P, 1024)   # d2 psum tile width
    NIW = P // IW
    return N, P, NB, IC, PC, PCW, IW, NIW


def build_nc(cfg):
    """Build the SPMD single-core program (same graph on all 8 cores)."""
    N, P, NB, IC, PC, PCW, IW, NIW = _cfg_derived(cfg)
    sign_jbs = set(cfg.get("sign_jbs", range(1, NB, 2)))

    nc = bacc.Bacc("TRN2", target_bir_lowering=False, debug=False)

    def din(name, shape, dt=BF16):
        return nc.dram_tensor(name, shape, dt, kind="ExternalInput")

    h2s = din("h2s", [DGD, P])
    h1s = din("h1s", [SA, P])
    djl = din("djl", [5, N])
    dri = din("dri", [5, P])
    gj = din("gj", [128, NB * 10])
    gfix = din("gfix", [128, IC * 10], F32)
    thrin = din("thrin", [128, 1], F32)
    identin = din("identin", [128, 128], F32)
    wdg1 = din("wdg1", [128, 8 * 256])
    bdg1 = din("bdg1", [128, 2], F32)
    wdg2 = din("wdg2", [128, 2 * 64])
    bdg2 = din("bdg2", [64, 1], F32)
    wdg3 = din("wdg3", [64, 32])
    bdg3 = din("bdg3", [32, 1], F32)
    edw1 = din("edw1", [3, 4])
    edb1 = din("edb1", [4, 1], F32)
    edw2 = din("edw2", [4, 4])
    edb2 = din("edb2", [4, 1], F32)
    w1a = din("w1a", [128, 512])
    w1b = din("w1b", [32, 512])
    w1c = din("w1c", [4, 512])
    b1t = din("b1t", [128, 4], F32)
    w2t = din("w2t", [128, 4 * 256])
    b2t = din("b2t", [128, 2], F32)
    w3t = din("w3t", [128, 2 * 128])
    b3t = din("b3t", [128, 1], F32)
    out = nc.dram_tensor("out", [CO, P], F32, kind="ExternalOutput")

    with tile.TileContext(nc) as tc:
        with (
            tc.tile_pool(name="const", bufs=1) as cpool,
            tc.tile_pool(name="big", bufs=1) as bpool,
            tc.tile_pool(name="dram", bufs=1, space="DRAM") as dpool,
        ):
            # ---- resident inputs -------------------------------------
            h2t = []
            for kb in range(8):
                t = bpool.tile([128, P], BF16, name=f"h2t{kb}")
                nc.sync.dma_start(t[:], h2s[kb * 128:(kb + 1) * 128, :])
                h2t.append(t)
            h1t = bpool.tile([128, P], BF16, name="h1t")
            nc.sync.dma_start(h1t[:], h1s[:])
            djlt = cpool.tile([5, N], BF16, name="djlt")
            nc.sync.dma_start(djlt[:], djl[:])
            drit = cpool.tile([5, P], BF16, name="drit")
            nc.sync.dma_start(drit[:], dri[:])
            gjt = cpool.tile([128, NB * 10], BF16, name="gjt")
            nc.sync.dma_start(gjt[:], gj[:])
            gfixt = cpool.tile([128, IC * 10], F32, name="gfixt")
            nc.sync.dma_start(gfixt[:], gfix[:])
            thrt = cpool.tile([128, 1], F32, name="thrt")
            nc.sync.dma_start(thrt[:], thrin[:])
            identt = cpool.tile([128, 128], F32, name="identt")
            nc.sync.dma_start(identt[:], identin[:])

            def loadw(name, src, shape, dt=BF16):
                t = cpool.tile(shape, dt, name=name)
                nc.sync.dma_start(t[:], src[:])
                return t

            _consts = {}

            def constap(val):
                if val not in _consts:
                    t = cpool.tile([128, 1], F32,
                                   name=f"cst{len(_consts)}")
                    nc.gpsimd.memset(t[:], val)
                    _consts[val] = t
                return _consts[val][:]

            wdg1t = loadw("wdg1t", wdg1, [128, 8 * 256])
            bdg1t = loadw("bdg1t", bdg1, [128, 2], F32)
            wdg2t = loadw("wdg2t", wdg2, [128, 2 * 64])
            bdg2t = loadw("bdg2t", bdg2, [64, 1], F32)
            wdg3t = loadw("wdg3t", wdg3, [64, 32])
            bdg3t = loadw("bdg3t", bdg3, [32, 1], F32)
            edw1t = loadw("edw1t", edw1, [3, 4])
            edb1t = loadw("edb1t", edb1, [4, 1], F32)
            edw2t = loadw("edw2t", edw2, [4, 4])
            edb2t = loadw("edb2t", edb2, [4, 1], F32)
            w1at = loadw("w1at", w1a, [128, 512])
            w1bt = loadw("w1bt", w1b, [32, 512])
            w1ct = loadw("w1ct", w1c, [4, 512])
            b1tt = loadw("b1tt", b1t, [128, 4], F32)
            w2tt = loadw("w2tt", w2t, [128, 4 * 256])
            b2tt = loadw("b2tt", b2t, [128, 2], F32)
            w3tt = loadw("w3tt", w3t, [128, 2 * 128])
            b3tt = loadw("b3tt", b3t, [128, 1], F32)

            # ---- phase B: mask + neighbor stats ----------------------
            # S accumulates in SBUF f32 on VectorE: straight blocks +1x,
            # Sign-form blocks +0.5x (with the 0.5*sum(G) shift folded
            # into gfix on the host).
            s_acc = bpool.tile([128, IC * 10], F32, name="s_acc")
            with (
                tc.tile_pool(name="spsum", bufs=2, space="PSUM") as spool,
                tc.tile_pool(name="maskp", bufs=3) as mpool,
            ):
                with tc.tile_pool(name="d2psum", bufs=3, space="PSUM") as d2pool:
                    for jb in range(NB):
                        maskt = mpool.tile([128, P], BF16, name="maskt", tag="mask")
                        for iw in range(NIW):
                            d2t = d2pool.tile([128, IW], F32, name="d2t", tag="d2")
                            for c5 in range(IW // 512):
                                lo = iw * IW + c5 * 512
                                nc.tensor.matmul(
                                    d2t[:, c5 * 512:(c5 + 1) * 512],
                                    djlt[:, jb * 128:(jb + 1) * 128],
                                    drit[:, lo:lo + 512],
                                    start=True, stop=True,
                                )
                            msl = maskt[:, iw * IW:(iw + 1) * IW]
                            if jb in sign_jbs:
                                # sign(thr - d2) in {-1,0,1}
                                nc.scalar.activation(
                                    msl, d2t[:], AF.Sign,
                                    bias=thrt[:], scale=-1.0,
                                )
                            else:
                                nc.vector.tensor_scalar(
                                    msl, d2t[:], thrt[:], None, ALU.is_lt)
                        sps = spool.tile([128, IC * 10], F32, name="sps",
                                         tag="sps")
                        for ic in range(IC):
                            nc.tensor.matmul(
                                sps[:, ic * 10:(ic + 1) * 10],
                                maskt[:, ic * 128:(ic + 1) * 128],
                                gjt[:, jb * 10:(jb + 1) * 10],
                                start=True, stop=True,
                            )
                        if jb == 0:
                            nc.vector.tensor_copy(s_acc[:], sps[:])
                        elif jb in sign_jbs:
                            nc.vector.scalar_tensor_tensor(
                                s_acc[:], sps[:], 0.5, s_acc[:],
                                op0=ALU.mult, op1=ALU.add)
                        else:
                            nc.vector.tensor_add(s_acc[:], s_acc[:], sps[:])

                # ---- phase C: S fixup into planes --------------------
                # planes layout [r, q*IC + c]
                planes = bpool.tile([128, 10 * IC], F32, name="planes")
                pl3 = planes.rearrange("p (q c) -> p q c", c=IC)
                for ic in range(IC):
                    ssl = slice(ic * 10, (ic + 1) * 10)
                    nc.vector.tensor_sub(
                        pl3[:, :, ic], s_acc[:, ssl], gfixt[:, ssl])

            # ---- phase D: closed-form 3x3 eigvalsh -------------------
            with tc.tile_pool(name="eig", bufs=1) as ep:
                def Pq(q):
                    return planes[:, q * IC:(q + 1) * IC]

                def T(name):
                    return ep.tile([128, IC], F32, name=name)

                v = nc.vector
                sc = nc.scalar
                n_, sx, sy, sz = Pq(0), Pq(1), Pq(2), Pq(3)
                sxx, sxy, sxz, syy, syz, szz = (Pq(i) for i in range(4, 10))
                ncl = T("ncl"); v.tensor_scalar_max(ncl[:], n_, 1.0)
                inv = T("inv"); v.reciprocal(inv[:], ncl[:])
                t0 = T("t0"); t1 = T("t1")
                cov = {}
                for nm, (a, b2_, s2ab) in dict(
                    xx=(sx, sx, sxx), xy=(sx, sy, sxy), xz=(sx, sz, sxz),
                    yy=(sy, sy, syy), yz=(sy, sz, syz), zz=(sz, sz, szz),
                ).items():
                    cab = T("c" + nm)
                    v.tensor_mul(t0[:], a, b2_)
                    v.tensor_mul(t0[:], t0[:], inv[:])
                    v.tensor_sub(cab[:], s2ab, t0[:])
                    cov[nm] = cab
                q_ = T("q_")
                v.tensor_add(q_[:], cov["xx"][:], cov["yy"][:])
                v.tensor_add(q_[:], q_[:], cov["zz"][:])
                v.tensor_scalar_mul(q_[:], q_[:], 1.0 / 3.0)
                for nm in ("xx", "yy", "zz"):
                    v.tensor_sub(cov[nm][:], cov[nm][:], q_[:])
                p2 = T("p2")
                v.tensor_mul(p2[:], cov["xx"][:], cov["xx"][:])
                v.tensor_mul(t0[:], cov["yy"][:], cov["yy"][:])
                v.tensor_add(p2[:], p2[:], t0[:])
                v.tensor_mul(t0[:], cov["zz"][:], cov["zz"][:])
                v.tensor_add(p2[:], p2[:], t0[:])
                v.tensor_mul(t1[:], cov["xy"][:], cov["xy"][:])
                v.tensor_mul(t0[:], cov["xz"][:], cov["xz"][:])
                v.tensor_add(t1[:], t1[:], t0[:])
                v.tensor_mul(t0[:], cov["yz"][:], cov["yz"][:])
                v.tensor_add(t1[:], t1[:], t0[:])
                v.scalar_tensor_tensor(
                    p2[:], t1[:], 2.0, p2[:], op0=ALU.mult, op1=ALU.add)
                v.tensor_scalar_max(p2[:], p2[:], 1e-30)
                pm = T("pm")
                sc.activation(pm[:], p2[:], AF.Sqrt, scale=1.0 / 6.0)
                ipm = T("ipm"); v.reciprocal(ipm[:], pm[:])
                for nm in cov:
                    v.tensor_mul(cov[nm][:], cov[nm][:], ipm[:])
                m1 = T("m1"); m2 = T("m2"); m3 = T("m3")
                v.tensor_mul(m1[:], cov["yy"][:], cov["zz"][:])
                v.tensor_mul(t0[:], cov["yz"][:], cov["yz"][:])
                v.tensor_sub(m1[:], m1[:], t0[:])
                v.tensor_mul(m2[:], cov["xy"][:], cov["zz"][:])
                v.tensor_mul(t0[:], cov["yz"][:], cov["xz"][:])
                v.tensor_sub(m2[:], m2[:], t0[:])
                v.tensor_mul(m3[:], cov["xy"][:], cov["yz"][:])
                v.tensor_mul(t0[:], cov["yy"][:], cov["xz"][:])
                v.tensor_sub(m3[:], m3[:], t0[:])
                det = T("det")
                v.tensor_mul(det[:], cov["xx"][:], m1[:])
                v.tensor_mul(t0[:], cov["xy"][:], m2[:])
                v.tensor_sub(det[:], det[:], t0[:])
                v.tensor_mul(t0[:], cov["xz"][:], m3[:])
                v.tensor_add(det[:], det[:], t0[:])
                r_ = T("r_")
                v.tensor_scalar(r_[:], det[:], 0.5, 1.0, ALU.mult, ALU.min)
                v.tensor_scalar_max(r_[:], r_[:], -1.0)
                # arccos(r) with arctan limited to [0, pi/4]
                ar = T("ar"); sc.activation(ar[:], r_[:], AF.Abs)
                sq = T("sq")
                v.tensor_mul(sq[:], r_[:], r_[:])
                v.tensor_scalar(sq[:], sq[:], -1.0, 1.0, ALU.mult, ALU.add)
                v.tensor_scalar_max(sq[:], sq[:], 0.0)
                sc.activation(sq[:], sq[:], AF.Sqrt)
                mn = T("mn"); mx = T("mx")
                v.tensor_tensor(mn[:], ar[:], sq[:], ALU.min)
                v.tensor_tensor(mx[:], ar[:], sq[:], ALU.max)
                imx = T("imx"); v.reciprocal(imx[:], mx[:])
                u_ = T("u_"); v.tensor_mul(u_[:], mn[:], imx[:])
                at = T("at"); sc.activation(at[:], u_[:], AF.Arctan)
                c1 = T("c1")
                v.tensor_tensor(c1[:], ar[:], sq[:], ALU.is_gt)
                mm_ = T("mm_")
                v.tensor_scalar(
                    mm_[:], at[:], 2.0, -math.pi / 2, ALU.mult, ALU.add)
                v.tensor_mul(mm_[:], mm_[:], c1[:])
                thp = T("thp")
                v.tensor_scalar(
                    thp[:], at[:], -1.0, math.pi / 2, ALU.mult, ALU.add)
                v.tensor_add(thp[:], thp[:], mm_[:])
                neg = T("neg")
                v.tensor_scalar(neg[:], r_[:], 0.0, None, ALU.is_lt)
                n2 = T("n2")
                v.tensor_scalar(
                    n2[:], thp[:], -2.0, math.pi, ALU.mult, ALU.add)
                v.tensor_mul(n2[:], n2[:], neg[:])
                th = T("th")
                v.tensor_add(th[:], thp[:], n2[:])
                # eigenvalues (ascending) into eigout [ls | lm | lb]
                eigout = bpool.tile([128, 3 * IC], F32, name="eigout")
                ls = eigout[:, 0:IC]
                lm = eigout[:, IC:2 * IC]
                lb = eigout[:, 2 * IC:3 * IC]
                cb = T("cb")
                sc.activation(cb[:], th[:], AF.Sin,
                              bias=constap(math.pi / 2), scale=1.0 / 3.0)
                ss = T("ss")
                sc.activation(ss[:], th[:], AF.Sin,
                              bias=constap(math.pi / 6), scale=1.0 / 3.0)
                v.tensor_mul(cb[:], cb[:], pm[:])
                v.tensor_mul(ss[:], ss[:], pm[:])
                v.scalar_tensor_tensor(
                    lb, cb[:], 2.0, q_[:], op0=ALU.mult, op1=ALU.add)
                v.scalar_tensor_tensor(
                    ls, ss[:], -2.0, q_[:], op0=ALU.mult, op1=ALU.add)
                v.tensor_scalar_mul(lm, q_[:], 3.0)
                v.tensor_sub(lm, lm, lb)
                v.tensor_sub(lm, lm, ls)

            # ---- phase E: transpose eig to channel-major + ED MLP ----
            with tc.tile_pool(name="cpsum", bufs=3, space="PSUM") as pp:
                trp = pp.tile([3 * IC, 128], F32, name="trp", tag="ps")
                nc.tensor.transpose(trp[:], eigout[:], identt[:])
                trsb = bpool.tile([3 * IC, 128], BF16, name="trsb")
                nc.vector.tensor_copy(trsb[:], trp[:])
                edram = dpool.tile([3 * IC, 128], BF16, name="edram")
                nc.sync.dma_start(edram[:], trsb[:])
                eigcm = bpool.tile([3, P], BF16, name="eigcm")
                nc.sync.dma_start(
                    eigcm[:], edram.rearrange("(q c) r -> q (c r)", q=3))

                zed = bpool.tile([4, P], BF16, name="zed")
                ed1 = bpool.tile([4, P], BF16, name="ed1")
                for pc in range(PC):
                    psl = slice(pc * PCW, (pc + 1) * PCW)
                    e1p = pp.tile([4, PCW], F32, name="e1p", tag="ps")
                    nc.tensor.matmul(e1p[:], edw1t[:], eigcm[:, psl],
                                     start=True, stop=True)
                    nc.scalar.activation(ed1[:, psl], e1p[:], AF.Relu,
                                         bias=edb1t[:])
                for pc in range(PC):
                    psl = slice(pc * PCW, (pc + 1) * PCW)
                    e2p = pp.tile([4, PCW], F32, name="e2p", tag="ps")
                    nc.tensor.matmul(e2p[:], edw2t[:], ed1[:, psl],
                                     start=True, stop=True)
                    nc.scalar.activation(zed[:, psl], e2p[:], AF.Identity,
                                         bias=edb2t[:])

                # ---- phase F: DG convs 1024->256->64->32 -------------
                dg1 = [bpool.tile([128, P], BF16, name=f"dg1_{m}")
                       for m in range(2)]
                for mo in range(2):
                    for pc in range(PC):
                        psl = slice(pc * PCW, (pc + 1) * PCW)
                        ps = pp.tile([128, PCW], F32, name="ps", tag="ps")
                        for kb in range(8):
                            nc.tensor.matmul(
                                ps[:],
                                wdg1t[:, kb * 256 + mo * 128:
                                      kb * 256 + (mo + 1) * 128],
                                h2t[kb][:, psl],
                                start=(kb == 0), stop=(kb == 7),
                            )
                        nc.scalar.activation(
                            dg1[mo][:, psl], ps[:], AF.Relu,
                            bias=bdg1t[:, mo:mo + 1])
                dg2 = bpool.tile([64, P], BF16, name="dg2")
                for pc in range(PC):
                    psl = slice(pc * PCW, (pc + 1) * PCW)
                    ps = pp.tile([64, PCW], F32, name="ps", tag="ps")
                    for kb in range(2):
                        nc.tensor.matmul(
                            ps[:], wdg2t[:, kb * 64:(kb + 1) * 64],
                            dg1[kb][:, psl],
                            start=(kb == 0), stop=(kb == 1))
                    nc.scalar.activation(dg2[:, psl], ps[:], AF.Relu,
                                         bias=bdg2t[:])
                zdg = bpool.tile([32, P], BF16, name="zdg")
                for pc in range(PC):
                    psl = slice(pc * PCW, (pc + 1) * PCW)
                    ps = pp.tile([32, PCW], F32, name="ps", tag="ps")
                    nc.tensor.matmul(ps[:], wdg3t[:], dg2[:, psl],
                                     start=True, stop=True)
                    nc.scalar.activation(zdg[:, psl], ps[:], AF.Relu,
                                         bias=bdg3t[:])

                # ---- phase G: final convs 164->512->256->128 ---------
                z1 = [bpool.tile([128, P], BF16, name=f"z1_{m}")
                      for m in range(4)]
                for mo in range(4):
                    for pc in range(PC):
                        psl = slice(pc * PCW, (pc + 1) * PCW)
                        ps = pp.tile([128, PCW], F32, name="ps", tag="ps")
                        msl = slice(mo * 128, (mo + 1) * 128)
                        nc.tensor.matmul(ps[:], w1at[:, msl], h1t[:, psl],
                                         start=True, stop=False)
                        nc.tensor.matmul(ps[:], w1bt[:, msl], zdg[:, psl],
                                         start=False, stop=False)
                        nc.tensor.matmul(ps[:], w1ct[:, msl], zed[:, psl],
                                         start=False, stop=True)
                        nc.scalar.activation(
                            z1[mo][:, psl], ps[:], AF.Relu,
                            bias=b1tt[:, mo:mo + 1])
                z2 = [bpool.tile([128, P], BF16, name=f"z2_{m}")
                      for m in range(2)]
                for mo in range(2):
                    for pc in range(PC):
                        psl = slice(pc * PCW, (pc + 1) * PCW)
                        ps = pp.tile([128, PCW], F32, name="ps", tag="ps")
                        for kb in range(4):
                            nc.tensor.matmul(
                                ps[:],
                                w2tt[:, kb * 256 + mo * 128:
                                     kb * 256 + (mo + 1) * 128],
                                z1[kb][:, psl],
                                start=(kb == 0), stop=(kb == 3))
                        nc.scalar.activation(
                            z2[mo][:, psl], ps[:], AF.Relu,
                            bias=b2tt[:, mo:mo + 1])
                outz = bpool.tile([128, P], F32, name="outz")
                for pc in range(PC):
                    psl = slice(pc * PCW, (pc + 1) * PCW)
                    ps = pp.tile([128, PCW], F32, name="ps", tag="ps")
                    for kb in range(2):
                        nc.tensor.matmul(
                            ps[:], w3tt[:, kb * 128:(kb + 1) * 128],
                            z2[kb][:, psl],
                            start=(kb == 0), stop=(kb == 1))
                    nc.scalar.activation(outz[:, psl], ps[:], AF.Relu,
                                         bias=b3tt[:])
                nc.sync.dma_start(out[:], outz[:])

    nc.compile()
    return nc


def _fib_directions(k=64):
    i = np.arange(k)
    phi = np.pi * (3.0 - np.sqrt(5.0)) * i
    ct = 1.0 - 2.0 * (i + 0.5) / k
    st = np.sqrt(np.maximum(1.0 - ct * ct, 0.0))
    u = np.stack([st * np.cos(phi), st * np.sin(phi), ct], -1)
    return np.concatenate([u, -u], 0)  # [2k, 3]


_DIRS = _fib_directions(64)


def host_prep(xyz, h1, h2_in, weights, cfg):
    """Build per-core in_maps. xyz/h1/h2_in are full f32 arrays."""
    N, P, NB, IC, PC, PCW, IW, NIW = _cfg_derived(cfg)
    sign_jbs = set(cfg.get("sign_jbs", range(1, NB, 2)))
    ncores = cfg.get("ncores", NCORES)
    nb_ = cfg.get("B", B)

    w = {k: np.asarray(v, np.float32) for k, v in weights.items()}
    # fold BN scale into weights, bias' = s*b + t
    def fold(wk, bk, sk, tk):
        W = (w[wk] * w[sk][:, None]).astype(np.float32)
        bias = (w[sk] * w[bk] + w[tk]).astype(np.float32)
        return W, bias

    Wdg1, bdg1 = fold("dg_w1", "dg_b1", "dg_s1", "dg_t1")
    Wdg2, bdg2 = fold("dg_w2", "dg_b2", "dg_s2", "dg_t2")
    Wdg3, bdg3 = fold("dg_w3", "dg_b3", "dg_s3", "dg_t3")
    W1, b1 = fold("w1", "b1", "s1", "t1")
    W2, b2 = fold("w2", "b2", "s2", "t2")
    W3, b3 = fold("w3", "b3", "s3", "t3")

    com = {}
    com["wdg1"] = (Wdg1.T.reshape(8, 128, 256).transpose(1, 0, 2)
                   .reshape(128, 8 * 256).astype(NP_BF16))
    com["bdg1"] = bdg1.reshape(2, 128).T.copy()
    com["wdg2"] = (Wdg2.T.reshape(2, 128, 64).transpose(1, 0, 2)
                   .reshape(128, 128).astype(NP_BF16))
    com["bdg2"] = bdg2.reshape(64, 1).copy()
    com["wdg3"] = Wdg3.T.astype(NP_BF16)
    com["bdg3"] = bdg3.reshape(32, 1).copy()
    com["edw1"] = (w["ed_w1"].T / NPTS).astype(NP_BF16)
    com["edb1"] = w["ed_b1"].reshape(4, 1).copy()
    com["edw2"] = w["ed_w2"].T.astype(NP_BF16)
    com["edb2"] = w["ed_b2"].reshape(4, 1).copy()
    W1T = W1.T  # [164, 512]
    com["w1a"] = W1T[0:128].astype(NP_BF16)
    com["w1b"] = W1T[128:160].astype(NP_BF16)
    com["w1c"] = W1T[160:164].astype(NP_BF16)
    com["b1t"] = b1.reshape(4, 128).T.copy()
    com["w2t"] = (W2.T.reshape(4, 128, 256).transpose(1, 0, 2)
                  .reshape(128, 4 * 256).astype(NP_BF16))
    com["b2t"] = b2.reshape(2, 128).T.copy()
    com["w3t"] = (W3.T.reshape(2, 128, 128).transpose(1, 0, 2)
                  .reshape(128, 2 * 128).astype(NP_BF16))
    com["b3t"] = b3.reshape(1, 128).T.copy()
    com["identin"] = np.eye(128, dtype=np.float32)

    in_maps = []
    for c in range(ncores):
        bidx, h = c // 2, c % 2
        sl = slice(h * P, (h + 1) * P)
        X = np.asarray(xyz[bidx], np.float32)[:N]
        X = X - X.mean(0, keepdims=True)
        x2 = (X * X).sum(-1)
        one = np.ones_like(x2)
        djl = np.stack([X[:, 0], X[:, 1], X[:, 2], x2, one])
        Xi, x2i = X[sl], x2[sl]
        dri = np.stack([-2 * Xi[:, 0], -2 * Xi[:, 1], -2 * Xi[:, 2],
                        np.ones(P, np.float32), x2i])
        G = np.concatenate(
            [one[:, None], X,
             X[:, [0, 0, 0, 1, 1, 2]] * X[:, [0, 1, 2, 1, 2, 2]]], 1)
        # [N, 10]: 1, x, y, z, xx, xy, xz, yy, yz, zz
        gj = (G.reshape(NB, 128, 10).transpose(1, 0, 2)
              .reshape(128, NB * 10))
        gsgn = np.zeros(10, np.float32)
        for jb in sign_jbs:
            gsgn += G[jb * 128:(jb + 1) * 128].sum(0)
        gfix = G[sl] - 0.5 * gsgn  # [P, 10]
        gfixt = (gfix.reshape(IC, 128, 10).transpose(1, 0, 2)
                 .reshape(128, IC * 10))
        proj = X @ _DIRS.T.astype(np.float32)  # [N, 128]
        pmax = proj.max(0)
        diam = float((pmax[:64] + pmax[64:]).max())
        thr = np.full((128, 1), (FACTOR * diam) ** 2, np.float32)

        m = dict(com)
        m["h2s"] = np.ascontiguousarray(
            np.asarray(h2_in[bidx], np.float32)[:, sl]).astype(NP_BF16)
        m["h1s"] = np.ascontiguousarray(
            np.asarray(h1[bidx], np.float32)[:, sl]).astype(NP_BF16)
        m["djl"] = djl.astype(NP_BF16)
        m["dri"] = dri.astype(NP_BF16)
        m["gj"] = gj.astype(NP_BF16)
        m["gfix"] = gfixt.astype(np.float32)
        m["thrin"] = thr
        in_maps.append(m)
    return in_maps


_NC_CACHE = {}


def _get_nc(cfg_key=None):
    if "nc" not in _NC_CACHE:
        _NC_CACHE["nc"] = build_nc(dict(FULL_CFG))
    return _NC_CACHE["nc"]


def kernel(**inputs):
    from concourse.bass_utils import run_bass_kernel_spmd

    xyz = np.asarray(inputs["xyz"], np.float32)
    h1 = np.asarray(inputs["h1"], np.float32)
    h2_in = np.asarray(inputs["h2_in"], np.float32)
    weights = {k: v for k, v in inputs.items()
               if k not in ("xyz", "h1", "h2_in")}

    nc = _get_nc()
    cfg = dict(FULL_CFG)
    in_maps = host_prep(xyz, h1, h2_in, weights, cfg)
    res = run_bass_kernel_spmd(nc, in_maps, core_ids=list(range(NCORES)))
    P = cfg["P"]
    z = np.empty((B, CO, NPTS), np.float32)
    for c in range(NCORES):
        bidx, h = c // 2, c % 2
        z[bidx, :, h * P:(h + 1) * P] = res.results[c]["out"]
    return (inputs["xyz"], z)


# revision 12
# speedup vs baseline: 1.2850x; 1.2850x over previous
"""Trainium2 Bass kernel for nn_Adaptive_EDDG (gnn_message_passing).

Sharding: 8 cores = 4 batches x 2 point-halves (pure data-parallel SPMD,
no collectives).  Each core owns P=2048 points (i) of one batch and the
full N=4096 neighbor set (j).

Device pipeline per core:
  - d2 tiles [128 j, P i] via K=5 bf16 matmul:  [x,y,z,|x|^2,1]_j^T @
    [-2x,-2y,-2z,1,|x|^2]_i  -> pairwise squared distances in PSUM.
  - threshold (d2 < radius^2) into a 0/1 bf16 mask; split across
    VectorE (is_lt) and ScalarE (Sign trick, fixed up linearly).
  - mask tiles used as matmul weights vs per-point moments G[j,0:10] =
    [1,x,y,z,xx,xy,xz,yy,yz,zz] -> neighbor stats cnt/S1/S2 per point.
  - closed-form symmetric 3x3 eigvalsh (trig method, range-safe arccos
    via arctan) on [128,16] point-major planes.
  - ED 3->4->4 MLP, DG 1024->256->64->32 convs, concat with h1,
    164->512->256->128 convs; BN folded into weights on host; bf16
    matmuls with fp32 accumulation.

The radius is max pairwise distance * 0.1; computed on host via a
128-direction projection diameter (>=98% exact; final output is
insensitive to radius at the 1e-5 level, measured).
"""

import math
import os
import sys

import numpy as np

try:
    import concourse.bacc as bacc  # noqa: F401
except Exception:  # pragma: no cover
    for _p in ("/opt/trn_rl_repo", "/root/.axon_site/_ro/trn_rl_repo"):
        if os.path.isdir(_p) and _p not in sys.path:
            sys.path.insert(0, _p)
    import concourse.bacc as bacc

import ml_dtypes
import concourse.bass as bass
import concourse.mybir as mybir
import concourse.tile as tile

BF16 = mybir.dt.bfloat16
F32 = mybir.dt.float32
AF = mybir.ActivationFunctionType
ALU = mybir.AluOpType
NP_BF16 = ml_dtypes.bfloat16

B, NPTS, SA, DGD, EDC, CO = 4, 4096, 128, 1024, 4, 128
FACTOR = 0.1
NCORES = 8

FULL_CFG = dict(N=NPTS, P=NPTS // 2)


def _cfg_derived(cfg):
    N, P = cfg["N"], cfg["P"]
    NB = N // 128       # j blocks
    IC = P // 128       # i chunks
    PC = max(P // 512, 1)  # point chunks for convs
    PCW = min(P, 512)
    IW = min(P, 1024)   # d2 psum tile width
    NIW = P // IW
    return N, P, NB, IC, PC, PCW, IW, NIW


def build_nc(cfg):
    """Build the SPMD single-core program (same graph on all 8 cores)."""
    N, P, NB, IC, PC, PCW, IW, NIW = _cfg_derived(cfg)
    sign_jbs = set(cfg.get("sign_jbs",
                       [j for j in range(NB) if j % 4 >= 2]))

    nc = bacc.Bacc("TRN2", target_bir_lowering=False, debug=False)

    def din(name, shape, dt=BF16):
        return nc.dram_tensor(name, shape, dt, kind="ExternalInput")

    h2s = din("h2s", [DGD, P])
    h1s = din("h1s", [SA, P])
    djl = din("djl", [5, N])
    dri = din("dri", [5, P])
    gj = din("gj", [128, NB * 10])
    gfix = din("gfix", [128, IC * 10], F32)
    thrin = din("thrin", [128, 1], F32)
    identin = din("identin", [128, 128], F32)
    wdg1 = din("wdg1", [128, 8 * 256])
    bdg1 = din("bdg1", [128, 2], F32)
    wdg2 = din("wdg2", [128, 2 * 64])
    bdg2 = din("bdg2", [64, 1], F32)
    wdg3 = din("wdg3", [64, 32])
    bdg3 = din("bdg3", [32, 1], F32)
    edw1 = din("edw1", [3, 4])
    edb1 = din("edb1", [4, 1], F32)
    edw2 = din("edw2", [4, 4])
    edb2 = din("edb2", [4, 1], F32)
    w1a = din("w1a", [128, 512])
    w1b = din("w1b", [32, 512])
    w1c = din("w1c", [4, 512])
    b1t = din("b1t", [128, 4], F32)
    w2t = din("w2t", [128, 4 * 256])
    b2t = din("b2t", [128, 2], F32)
    w3t = din("w3t", [128, 2 * 128])
    b3t = din("b3t", [128, 1], F32)
    out = nc.dram_tensor("out", [CO, P], F32, kind="ExternalOutput")

    with tile.TileContext(nc) as tc:
        with (
            tc.tile_pool(name="const", bufs=1) as cpool,
            tc.tile_pool(name="big", bufs=1) as bpool,
            tc.tile_pool(name="dram", bufs=1, space="DRAM") as dpool,
        ):
            # ---- resident inputs (mask-phase inputs FIRST so the PE
            # can start d2 matmuls while the big DMAs stream in) ------
            djlt = cpool.tile([5, N], BF16, name="djlt")
            nc.sync.dma_start(djlt[:], djl[:])
            drit = cpool.tile([5, P], BF16, name="drit")
            nc.sync.dma_start(drit[:], dri[:])
            gjt = cpool.tile([128, NB * 10], BF16, name="gjt")
            nc.sync.dma_start(gjt[:], gj[:])
            thrt = cpool.tile([128, 1], F32, name="thrt")
            nc.sync.dma_start(thrt[:], thrin[:])
            gfixt = cpool.tile([128, IC * 10], F32, name="gfixt")
            nc.sync.dma_start(gfixt[:], gfix[:])
            identt = cpool.tile([128, 128], F32, name="identt")
            nc.sync.dma_start(identt[:], identin[:])
            h2t = []
            for kb in range(8):
                t = bpool.tile([128, P], BF16, name=f"h2t{kb}")
                nc.gpsimd.dma_start(t[:], h2s[kb * 128:(kb + 1) * 128, :])
                h2t.append(t)
            h1t = bpool.tile([128, P], BF16, name="h1t")
            nc.gpsimd.dma_start(h1t[:], h1s[:])

            def loadw(name, src, shape, dt=BF16):
                t = cpool.tile(shape, dt, name=name)
                nc.sync.dma_start(t[:], src[:])
                return t

            _consts = {}

            def constap(val):
                if val not in _consts:
                    t = cpool.tile([128, 1], F32,
                                   name=f"cst{len(_consts)}")
                    nc.gpsimd.memset(t[:], val)
                    _consts[val] = t
                return _consts[val][:]

            wdg1t = loadw("wdg1t", wdg1, [128, 8 * 256])
            bdg1t = loadw("bdg1t", bdg1, [128, 2], F32)
            wdg2t = loadw("wdg2t", wdg2, [128, 2 * 64])
            bdg2t = loadw("bdg2t", bdg2, [64, 1], F32)
            wdg3t = loadw("wdg3t", wdg3, [64, 32])
            bdg3t = loadw("bdg3t", bdg3, [32, 1], F32)
            edw1t = loadw("edw1t", edw1, [3, 4])
            edb1t = loadw("edb1t", edb1, [4, 1], F32)
            edw2t = loadw("edw2t", edw2, [4, 4])
            edb2t = loadw("edb2t", edb2, [4, 1], F32)
            w1at = loadw("w1at", w1a, [128, 512])
            w1bt = loadw("w1bt", w1b, [32, 512])
            w1ct = loadw("w1ct", w1c, [4, 512])
            b1tt = loadw("b1tt", b1t, [128, 4], F32)
            w2tt = loadw("w2tt", w2t, [128, 4 * 256])
            b2tt = loadw("b2tt", b2t, [128, 2], F32)
            w3tt = loadw("w3tt", w3t, [128, 2 * 128])
            b3tt = loadw("b3tt", b3t, [128, 1], F32)

            # ---- phase B: mask + neighbor stats ----------------------
            # jb quads: {4k,4k+1} cmp on VectorE (is_lt, 0/1 mask),
            # {4k+2,4k+3} cmp on ScalarE (Sign form, weighted 0.5 with
            # the 0.5*sum(G) shift folded into gfix on the host).
            # S-matmuls for a same-kind jb pair share one PSUM
            # accumulation, halving the VectorE accumulate chain.
            s_acc = bpool.tile([128, IC * 10], F32, name="s_acc")
            with (
                tc.tile_pool(name="spsum", bufs=2, space="PSUM") as spool,
                tc.tile_pool(name="maskp", bufs=4) as mpool,
            ):
                with tc.tile_pool(name="d2psum", bufs=2, space="PSUM") as d2pool:
                    for pjb in range(NB // 2):
                        pair = (2 * pjb, 2 * pjb + 1)
                        is_sign = pair[0] in sign_jbs
                        masks = {}
                        for jb in pair:
                            assert (jb in sign_jbs) == is_sign
                            maskt = mpool.tile([128, P], BF16,
                                               name="maskt", tag="mask")
                            masks[jb] = maskt
                            for iw in range(NIW):
                                d2t = d2pool.tile([128, IW], F32,
                                                  name="d2t", tag="d2")
                                for c5 in range(IW // 512):
                                    lo = iw * IW + c5 * 512
                                    nc.tensor.matmul(
                                        d2t[:, c5 * 512:(c5 + 1) * 512],
                                        djlt[:, jb * 128:(jb + 1) * 128],
                                        drit[:, lo:lo + 512],
                                        start=True, stop=True,
                                    )
                                msl = maskt[:, iw * IW:(iw + 1) * IW]
                                if is_sign:
                                    nc.scalar.activation(
                                        msl, d2t[:], AF.Sign,
                                        bias=thrt[:], scale=-1.0,
                                    )
                                else:
                                    nc.vector.tensor_scalar(
                                        msl, d2t[:], thrt[:], None,
                                        ALU.is_lt)
                        sps = spool.tile([128, IC * 10], F32, name="sps",
                                         tag="sps")
                        for ic in range(IC):
                            for ji, jb in enumerate(pair):
                                nc.tensor.matmul(
                                    sps[:, ic * 10:(ic + 1) * 10],
                                    masks[jb][:, ic * 128:(ic + 1) * 128],
                                    gjt[:, jb * 10:(jb + 1) * 10],
                                    start=(ji == 0), stop=(ji == 1),
                                )
                        if pjb == 0:
                            nc.vector.tensor_copy(s_acc[:], sps[:])
                        elif is_sign:
                            nc.vector.scalar_tensor_tensor(
                                s_acc[:], sps[:], 0.5, s_acc[:],
                                op0=ALU.mult, op1=ALU.add)
                        else:
                            nc.vector.tensor_add(s_acc[:], s_acc[:], sps[:])

                # ---- phase C: S fixup into planes --------------------
                # planes layout [r, q*IC + c]
                planes = bpool.tile([128, 10 * IC], F32, name="planes")
                pl3 = planes.rearrange("p (q c) -> p q c", c=IC)
                for ic in range(IC):
                    ssl = slice(ic * 10, (ic + 1) * 10)
                    nc.vector.tensor_sub(
                        pl3[:, :, ic], s_acc[:, ssl], gfixt[:, ssl])

            # epilogue helper: relu(psum + bias) on alternating engines
            _epi_n = [0]

            def epilogue(dst, ps, bias_ap, relu=True):
                _epi_n[0] += 1
                if _epi_n[0] % 2 == 0:
                    nc.scalar.activation(
                        dst, ps, AF.Relu if relu else AF.Identity,
                        bias=bias_ap)
                elif relu:
                    nc.vector.tensor_scalar(
                        dst, ps, bias_ap, 0.0, ALU.add, ALU.max)
                else:
                    nc.vector.tensor_scalar(
                        dst, ps, bias_ap, None, ALU.add)

            # ---- phase F: DG convs 1024->256->64->32 (emitted BEFORE
            # the eigensolver chain so the PE has queued work) ---------
            with tc.tile_pool(name="cpsum2", bufs=4, space="PSUM") as pp:
                dg1 = [bpool.tile([128, P], BF16, name=f"dg1_{m}")
                       for m in range(2)]
                for mo in range(2):
                    for pc in range(PC):
                        psl = slice(pc * PCW, (pc + 1) * PCW)
                        ps = pp.tile([128, PCW], F32, name="ps", tag="ps")
                        for kb in range(8):
                            nc.tensor.matmul(
                                ps[:],
                                wdg1t[:, kb * 256 + mo * 128:
                                      kb * 256 + (mo + 1) * 128],
                                h2t[kb][:, psl],
                                start=(kb == 0), stop=(kb == 7),
                            )
                        epilogue(dg1[mo][:, psl], ps[:],
                                 bdg1t[:, mo:mo + 1])
                dg2 = bpool.tile([64, P], BF16, name="dg2")
                for pc in range(PC):
                    psl = slice(pc * PCW, (pc + 1) * PCW)
                    ps = pp.tile([64, PCW], F32, name="ps", tag="ps")
                    for kb in range(2):
                        nc.tensor.matmul(
                            ps[:], wdg2t[:, kb * 64:(kb + 1) * 64],
                            dg1[kb][:, psl],
                            start=(kb == 0), stop=(kb == 1))
                    epilogue(dg2[:, psl], ps[:], bdg2t[:])
                zdg = bpool.tile([32, P], BF16, name="zdg")
                for pc in range(PC):
                    psl = slice(pc * PCW, (pc + 1) * PCW)
                    ps = pp.tile([32, PCW], F32, name="ps", tag="ps")
                    nc.tensor.matmul(ps[:], wdg3t[:], dg2[:, psl],
                                     start=True, stop=True)
                    epilogue(zdg[:, psl], ps[:], bdg3t[:])

            # ---- phase D: closed-form 3x3 eigvalsh -------------------
            with tc.tile_pool(name="eig", bufs=1) as ep:
                def Pq(q):
                    return planes[:, q * IC:(q + 1) * IC]

                def T(name):
                    return ep.tile([128, IC], F32, name=name)

                v = nc.vector
                sc = nc.scalar
                n_, sx, sy, sz = Pq(0), Pq(1), Pq(2), Pq(3)
                sxx, sxy, sxz, syy, syz, szz = (Pq(i) for i in range(4, 10))
                ncl = T("ncl"); v.tensor_scalar_max(ncl[:], n_, 1.0)
                inv = T("inv"); v.reciprocal(inv[:], ncl[:])
                t0 = T("t0"); t1 = T("t1")
                cov = {}
                for nm, (a, b2_, s2ab) in dict(
                    xx=(sx, sx, sxx), xy=(sx, sy, sxy), xz=(sx, sz, sxz),
                    yy=(sy, sy, syy), yz=(sy, sz, syz), zz=(sz, sz, szz),
                ).items():
                    cab = T("c" + nm)
                    v.tensor_mul(t0[:], a, b2_)
                    v.tensor_mul(t0[:], t0[:], inv[:])
                    v.tensor_sub(cab[:], s2ab, t0[:])
                    cov[nm] = cab
                q_ = T("q_")
                v.tensor_add(q_[:], cov["xx"][:], cov["yy"][:])
                v.tensor_add(q_[:], q_[:], cov["zz"][:])
                v.tensor_scalar_mul(q_[:], q_[:], 1.0 / 3.0)
                for nm in ("xx", "yy", "zz"):
                    v.tensor_sub(cov[nm][:], cov[nm][:], q_[:])
                p2 = T("p2")
                v.tensor_mul(p2[:], cov["xx"][:], cov["xx"][:])
                v.tensor_mul(t0[:], cov["yy"][:], cov["yy"][:])
                v.tensor_add(p2[:], p2[:], t0[:])
                v.tensor_mul(t0[:], cov["zz"][:], cov["zz"][:])
                v.tensor_add(p2[:], p2[:], t0[:])
                v.tensor_mul(t1[:], cov["xy"][:], cov["xy"][:])
                v.tensor_mul(t0[:], cov["xz"][:], cov["xz"][:])
                v.tensor_add(t1[:], t1[:], t0[:])
                v.tensor_mul(t0[:], cov["yz"][:], cov["yz"][:])
                v.tensor_add(t1[:], t1[:], t0[:])
                v.scalar_tensor_tensor(
                    p2[:], t1[:], 2.0, p2[:], op0=ALU.mult, op1=ALU.add)
                v.tensor_scalar_max(p2[:], p2[:], 1e-30)
                pm = T("pm")
                sc.activation(pm[:], p2[:], AF.Sqrt, scale=1.0 / 6.0)
                ipm = T("ipm"); v.reciprocal(ipm[:], pm[:])
                for nm in cov:
                    v.tensor_mul(cov[nm][:], cov[nm][:], ipm[:])
                m1 = T("m1"); m2 = T("m2"); m3 = T("m3")
                v.tensor_mul(m1[:], cov["yy"][:], cov["zz"][:])
                v.tensor_mul(t0[:], cov["yz"][:], cov["yz"][:])
                v.tensor_sub(m1[:], m1[:], t0[:])
                v.tensor_mul(m2[:], cov["xy"][:], cov["zz"][:])
                v.tensor_mul(t0[:], cov["yz"][:], cov["xz"][:])
                v.tensor_sub(m2[:], m2[:], t0[:])
                v.tensor_mul(m3[:], cov["xy"][:], cov["yz"][:])
                v.tensor_mul(t0[:], cov["yy"][:], cov["xz"][:])
                v.tensor_sub(m3[:], m3[:], t0[:])
                det = T("det")
                v.tensor_mul(det[:], cov["xx"][:], m1[:])
                v.tensor_mul(t0[:], cov["xy"][:], m2[:])
                v.tensor_sub(det[:], det[:], t0[:])
                v.tensor_mul(t0[:], cov["xz"][:], m3[:])
                v.tensor_add(det[:], det[:], t0[:])
                r_ = T("r_")
                v.tensor_scalar(r_[:], det[:], 0.5, 1.0, ALU.mult, ALU.min)
                v.tensor_scalar_max(r_[:], r_[:], -1.0)
                # arccos(r) with arctan limited to [0, pi/4]
                ar = T("ar"); sc.activation(ar[:], r_[:], AF.Abs)
                sq = T("sq")
                v.tensor_mul(sq[:], r_[:], r_[:])
                v.tensor_scalar(sq[:], sq[:], -1.0, 1.0, ALU.mult, ALU.add)
                v.tensor_scalar_max(sq[:], sq[:], 0.0)
                sc.activation(sq[:], sq[:], AF.Sqrt)
                mn = T("mn"); mx = T("mx")
                v.tensor_tensor(mn[:], ar[:], sq[:], ALU.min)
                v.tensor_tensor(mx[:], ar[:], sq[:], ALU.max)
                imx = T("imx"); v.reciprocal(imx[:], mx[:])
                u_ = T("u_"); v.tensor_mul(u_[:], mn[:], imx[:])
                at = T("at"); sc.activation(at[:], u_[:], AF.Arctan)
                c1 = T("c1")
                v.tensor_tensor(c1[:], ar[:], sq[:], ALU.is_gt)
                mm_ = T("mm_")
                v.tensor_scalar(
                    mm_[:], at[:], 2.0, -math.pi / 2, ALU.mult, ALU.add)
                v.tensor_mul(mm_[:], mm_[:], c1[:])
                thp = T("thp")
                v.tensor_scalar(
                    thp[:], at[:], -1.0, math.pi / 2, ALU.mult, ALU.add)
                v.tensor_add(thp[:], thp[:], mm_[:])
                neg = T("neg")
                v.tensor_scalar(neg[:], r_[:], 0.0, None, ALU.is_lt)
                n2 = T("n2")
                v.tensor_scalar(
                    n2[:], thp[:], -2.0, math.pi, ALU.mult, ALU.add)
                v.tensor_mul(n2[:], n2[:], neg[:])
                th = T("th")
                v.tensor_add(th[:], thp[:], n2[:])
                # eigenvalues (ascending) into eigout [ls | lm | lb]
                eigout = bpool.tile([128, 3 * IC], F32, name="eigout")
                ls = eigout[:, 0:IC]
                lm = eigout[:, IC:2 * IC]
                lb = eigout[:, 2 * IC:3 * IC]
                cb = T("cb")
                sc.activation(cb[:], th[:], AF.Sin,
                              bias=constap(math.pi / 2), scale=1.0 / 3.0)
                ss = T("ss")
                sc.activation(ss[:], th[:], AF.Sin,
                              bias=constap(math.pi / 6), scale=1.0 / 3.0)
                v.tensor_mul(cb[:], cb[:], pm[:])
                v.tensor_mul(ss[:], ss[:], pm[:])
                v.scalar_tensor_tensor(
                    lb, cb[:], 2.0, q_[:], op0=ALU.mult, op1=ALU.add)
                v.scalar_tensor_tensor(
                    ls, ss[:], -2.0, q_[:], op0=ALU.mult, op1=ALU.add)
                v.tensor_scalar_mul(lm, q_[:], 3.0)
                v.tensor_sub(lm, lm, lb)
                v.tensor_sub(lm, lm, ls)

            # ---- phase E: transpose eig to channel-major + ED MLP ----
            with tc.tile_pool(name="cpsum", bufs=3, space="PSUM") as pp:
                trp = pp.tile([3 * IC, 128], F32, name="trp", tag="ps")
                nc.tensor.transpose(trp[:], eigout[:], identt[:])
                trsb = bpool.tile([3 * IC, 128], BF16, name="trsb")
                nc.vector.tensor_copy(trsb[:], trp[:])
                edram = dpool.tile([3 * IC, 128], BF16, name="edram")
                nc.sync.dma_start(edram[:], trsb[:])
                eigcm = bpool.tile([3, P], BF16, name="eigcm")
                nc.sync.dma_start(
                    eigcm[:], edram.rearrange("(q c) r -> q (c r)", q=3))

                zed = bpool.tile([4, P], BF16, name="zed")
                ed1 = bpool.tile([4, P], BF16, name="ed1")
                for pc in range(PC):
                    psl = slice(pc * PCW, (pc + 1) * PCW)
                    e1p = pp.tile([4, PCW], F32, name="e1p", tag="ps")
                    nc.tensor.matmul(e1p[:], edw1t[:], eigcm[:, psl],
                                     start=True, stop=True)
                    nc.scalar.activation(ed1[:, psl], e1p[:], AF.Relu,
                                         bias=edb1t[:])
                for pc in range(PC):
                    psl = slice(pc * PCW, (pc + 1) * PCW)
                    e2p = pp.tile([4, PCW], F32, name="e2p", tag="ps")
                    nc.tensor.matmul(e2p[:], edw2t[:], ed1[:, psl],
                                     start=True, stop=True)
                    nc.scalar.activation(zed[:, psl], e2p[:], AF.Identity,
                                         bias=edb2t[:])

                # ---- phase G: final convs 164->512->256->128 ---------
                z1 = [bpool.tile([128, P], BF16, name=f"z1_{m}")
                      for m in range(4)]
                for mo in range(4):
                    for pc in range(PC):
                        psl = slice(pc * PCW, (pc + 1) * PCW)
                        ps = pp.tile([128, PCW], F32, name="ps", tag="ps")
                        msl = slice(mo * 128, (mo + 1) * 128)
                        nc.tensor.matmul(ps[:], w1at[:, msl], h1t[:, psl],
                                         start=True, stop=False)
                        nc.tensor.matmul(ps[:], w1bt[:, msl], zdg[:, psl],
                                         start=False, stop=False)
                        nc.tensor.matmul(ps[:], w1ct[:, msl], zed[:, psl],
                                         start=False, stop=True)
                        epilogue(z1[mo][:, psl], ps[:],
                                 b1tt[:, mo:mo + 1])
                z2 = [bpool.tile([128, P], BF16, name=f"z2_{m}")
                      for m in range(2)]
                for mo in range(2):
                    for pc in range(PC):
                        psl = slice(pc * PCW, (pc + 1) * PCW)
                        ps = pp.tile([128, PCW], F32, name="ps", tag="ps")
                        for kb in range(4):
                            nc.tensor.matmul(
                                ps[:],
                                w2tt[:, kb * 256 + mo * 128:
                                     kb * 256 + (mo + 1) * 128],
                                z1[kb][:, psl],
                                start=(kb == 0), stop=(kb == 3))
                        epilogue(z2[mo][:, psl], ps[:],
                                 b2tt[:, mo:mo + 1])
                outz = bpool.tile([128, P], F32, name="outz")
                for pc in range(PC):
                    psl = slice(pc * PCW, (pc + 1) * PCW)
                    ps = pp.tile([128, PCW], F32, name="ps", tag="ps")
                    for kb in range(2):
                        nc.tensor.matmul(
                            ps[:], w3tt[:, kb * 128:(kb + 1) * 128],
                            z2[kb][:, psl],
                            start=(kb == 0), stop=(kb == 1))
                    epilogue(outz[:, psl], ps[:], b3tt[:])
                nc.sync.dma_start(out[:], outz[:])

    nc.compile()
    return nc


def _fib_directions(k=64):
    i = np.arange(k)
    phi = np.pi * (3.0 - np.sqrt(5.0)) * i
    ct = 1.0 - 2.0 * (i + 0.5) / k
    st = np.sqrt(np.maximum(1.0 - ct * ct, 0.0))
    u = np.stack([st * np.cos(phi), st * np.sin(phi), ct], -1)
    return np.concatenate([u, -u], 0)  # [2k, 3]


_DIRS = _fib_directions(64)


def host_prep(xyz, h1, h2_in, weights, cfg):
    """Build per-core in_maps. xyz/h1/h2_in are full f32 arrays."""
    N, P, NB, IC, PC, PCW, IW, NIW = _cfg_derived(cfg)
    sign_jbs = set(cfg.get("sign_jbs",
                       [j for j in range(NB) if j % 4 >= 2]))
    ncores = cfg.get("ncores", NCORES)
    nb_ = cfg.get("B", B)

    w = {k: np.asarray(v, np.float32) for k, v in weights.items()}
    # fold BN scale into weights, bias' = s*b + t
    def fold(wk, bk, sk, tk):
        W = (w[wk] * w[sk][:, None]).astype(np.float32)
        bias = (w[sk] * w[bk] + w[tk]).astype(np.float32)
        return W, bias

    Wdg1, bdg1 = fold("dg_w1", "dg_b1", "dg_s1", "dg_t1")
    Wdg2, bdg2 = fold("dg_w2", "dg_b2", "dg_s2", "dg_t2")
    Wdg3, bdg3 = fold("dg_w3", "dg_b3", "dg_s3", "dg_t3")
    W1, b1 = fold("w1", "b1", "s1", "t1")
    W2, b2 = fold("w2", "b2", "s2", "t2")
    W3, b3 = fold("w3", "b3", "s3", "t3")

    com = {}
    com["wdg1"] = (Wdg1.T.reshape(8, 128, 256).transpose(1, 0, 2)
                   .reshape(128, 8 * 256).astype(NP_BF16))
    com["bdg1"] = bdg1.reshape(2, 128).T.copy()
    com["wdg2"] = (Wdg2.T.reshape(2, 128, 64).transpose(1, 0, 2)
                   .reshape(128, 128).astype(NP_BF16))
    com["bdg2"] = bdg2.reshape(64, 1).copy()
    com["wdg3"] = Wdg3.T.astype(NP_BF16)
    com["bdg3"] = bdg3.reshape(32, 1).copy()
    com["edw1"] = (w["ed_w1"].T / NPTS).astype(NP_BF16)
    com["edb1"] = w["ed_b1"].reshape(4, 1).copy()
    com["edw2"] = w["ed_w2"].T.astype(NP_BF16)
    com["edb2"] = w["ed_b2"].reshape(4, 1).copy()
    W1T = W1.T  # [164, 512]
    com["w1a"] = W1T[0:128].astype(NP_BF16)
    com["w1b"] = W1T[128:160].astype(NP_BF16)
    com["w1c"] = W1T[160:164].astype(NP_BF16)
    com["b1t"] = b1.reshape(4, 128).T.copy()
    com["w2t"] = (W2.T.reshape(4, 128, 256).transpose(1, 0, 2)
                  .reshape(128, 4 * 256).astype(NP_BF16))
    com["b2t"] = b2.reshape(2, 128).T.copy()
    com["w3t"] = (W3.T.reshape(2, 128, 128).transpose(1, 0, 2)
                  .reshape(128, 2 * 128).astype(NP_BF16))
    com["b3t"] = b3.reshape(1, 128).T.copy()
    com["identin"] = np.eye(128, dtype=np.float32)

    in_maps = []
    for c in range(ncores):
        bidx, h = c // 2, c % 2
        sl = slice(h * P, (h + 1) * P)
        X = np.asarray(xyz[bidx], np.float32)[:N]
        X = X - X.mean(0, keepdims=True)
        x2 = (X * X).sum(-1)
        one = np.ones_like(x2)
        djl = np.stack([X[:, 0], X[:, 1], X[:, 2], x2, one])
        Xi, x2i = X[sl], x2[sl]
        dri = np.stack([-2 * Xi[:, 0], -2 * Xi[:, 1], -2 * Xi[:, 2],
                        np.ones(P, np.float32), x2i])
        G = np.concatenate(
            [one[:, None], X,
             X[:, [0, 0, 0, 1, 1, 2]] * X[:, [0, 1, 2, 1, 2, 2]]], 1)
        # [N, 10]: 1, x, y, z, xx, xy, xz, yy, yz, zz
        gj = (G.reshape(NB, 128, 10).transpose(1, 0, 2)
              .reshape(128, NB * 10))
        gsgn = np.zeros(10, np.float32)
        for jb in sign_jbs:
            gsgn += G[jb * 128:(jb + 1) * 128].sum(0)
        gfix = G[sl] - 0.5 * gsgn  # [P, 10]
        gfixt = (gfix.reshape(IC, 128, 10).transpose(1, 0, 2)
                 .reshape(128, IC * 10))
        proj = X @ _DIRS.T.astype(np.float32)  # [N, 128]
        pmax = proj.max(0)
        diam = float((pmax[:64] + pmax[64:]).max())
        thr = np.full((128, 1), (FACTOR * diam) ** 2, np.float32)

        m = dict(com)
        m["h2s"] = np.ascontiguousarray(
            np.asarray(h2_in[bidx], np.float32)[:, sl]).astype(NP_BF16)
        m["h1s"] = np.ascontiguousarray(
            np.asarray(h1[bidx], np.float32)[:, sl]).astype(NP_BF16)
        m["djl"] = djl.astype(NP_BF16)
        m["dri"] = dri.astype(NP_BF16)
        m["gj"] = gj.astype(NP_BF16)
        m["gfix"] = gfixt.astype(np.float32)
        m["thrin"] = thr
        in_maps.append(m)
    return in_maps


_NC_CACHE = {}


def _get_nc(cfg_key=None):
    if "nc" not in _NC_CACHE:
        _NC_CACHE["nc"] = build_nc(dict(FULL_CFG))
    return _NC_CACHE["nc"]


def kernel(**inputs):
    from concourse.bass_utils import run_bass_kernel_spmd

    xyz = np.asarray(inputs["xyz"], np.float32)
    h1 = np.asarray(inputs["h1"], np.float32)
    h2_in = np.asarray(inputs["h2_in"], np.float32)
    weights = {k: v for k, v in inputs.items()
               if k not in ("xyz", "h1", "h2_in")}

    nc = _get_nc()
    cfg = dict(FULL_CFG)
    in_maps = host_prep(xyz, h1, h2_in, weights, cfg)
    res = run_bass_kernel_spmd(nc, in_maps, core_ids=list(range(NCORES)))
    P = cfg["P"]
    z = np.empty((B, CO, NPTS), np.float32)
    for c in range(NCORES):
        bidx, h = c // 2, c % 2
        z[bidx, :, h * P:(h + 1) * P] = res.results[c]["out"]
    return (inputs["xyz"], z)


# revision 14
# speedup vs baseline: 1.2993x; 1.0111x over previous
"""Trainium2 Bass kernel for nn_Adaptive_EDDG (gnn_message_passing).

Sharding: 8 cores = 4 batches x 2 point-halves (pure data-parallel SPMD,
no collectives).  Each core owns P=2048 points (i) of one batch and the
full N=4096 neighbor set (j).

Device pipeline per core:
  - d2 tiles [128 j, P i] via K=5 bf16 matmul:  [x,y,z,|x|^2,1]_j^T @
    [-2x,-2y,-2z,1,|x|^2]_i  -> pairwise squared distances in PSUM.
  - threshold (d2 < radius^2) into a 0/1 bf16 mask; split across
    VectorE (is_lt) and ScalarE (Sign trick, fixed up linearly).
  - mask tiles used as matmul weights vs per-point moments G[j,0:10] =
    [1,x,y,z,xx,xy,xz,yy,yz,zz] -> neighbor stats cnt/S1/S2 per point.
  - closed-form symmetric 3x3 eigvalsh (trig method, range-safe arccos
    via arctan) on [128,16] point-major planes.
  - ED 3->4->4 MLP, DG 1024->256->64->32 convs, concat with h1,
    164->512->256->128 convs; BN folded into weights on host; bf16
    matmuls with fp32 accumulation.

The radius is max pairwise distance * 0.1; computed on host via a
128-direction projection diameter (>=98% exact; final output is
insensitive to radius at the 1e-5 level, measured).
"""

import math
import os
import sys

import numpy as np

try:
    import concourse.bacc as bacc  # noqa: F401
except Exception:  # pragma: no cover
    for _p in ("/opt/trn_rl_repo", "/root/.axon_site/_ro/trn_rl_repo"):
        if os.path.isdir(_p) and _p not in sys.path:
            sys.path.insert(0, _p)
    import concourse.bacc as bacc

import ml_dtypes
import concourse.bass as bass
import concourse.mybir as mybir
import concourse.tile as tile

BF16 = mybir.dt.bfloat16
F32 = mybir.dt.float32
AF = mybir.ActivationFunctionType
ALU = mybir.AluOpType
NP_BF16 = ml_dtypes.bfloat16

B, NPTS, SA, DGD, EDC, CO = 4, 4096, 128, 1024, 4, 128
FACTOR = 0.1
NCORES = 8

FULL_CFG = dict(N=NPTS, P=NPTS // 2)


def _cfg_derived(cfg):
    N, P = cfg["N"], cfg["P"]
    NB = N // 128       # j blocks
    IC = P // 128       # i chunks
    PC = max(P // 512, 1)  # point chunks for convs
    PCW = min(P, 512)
    IW = min(P, 1024)   # d2 psum tile width
    NIW = P // IW
    return N, P, NB, IC, PC, PCW, IW, NIW


def build_nc(cfg):
    """Build the SPMD single-core program (same graph on all 8 cores)."""
    N, P, NB, IC, PC, PCW, IW, NIW = _cfg_derived(cfg)
    sign_jbs = set(cfg.get("sign_jbs",
                       [j for j in range(NB) if j % 4 >= 2]))

    nc = bacc.Bacc("TRN2", target_bir_lowering=False, debug=False)

    def din(name, shape, dt=BF16):
        return nc.dram_tensor(name, shape, dt, kind="ExternalInput")

    h2s = din("h2s", [DGD // 2, 2 * P], mybir.dt.float8e4)
    h1s = din("h1s", [SA, P])
    djl = din("djl", [5, N])
    dri = din("dri", [5, P])
    gj = din("gj", [128, NB * 10])
    gfix = din("gfix", [128, IC * 10], F32)
    thrin = din("thrin", [128, 1], F32)
    identin = din("identin", [128, 128], F32)
    wdg1 = din("wdg1", [128, 4 * 2 * 256], mybir.dt.float8e4)
    bdg1 = din("bdg1", [128, 2], F32)
    wdg2 = din("wdg2", [128, 2 * 64])
    bdg2 = din("bdg2", [64, 1], F32)
    wdg3 = din("wdg3", [64, 32])
    bdg3 = din("bdg3", [32, 1], F32)
    edw1 = din("edw1", [3, 4])
    edb1 = din("edb1", [4, 1], F32)
    edw2 = din("edw2", [4, 4])
    edb2 = din("edb2", [4, 1], F32)
    w1a = din("w1a", [128, 512])
    w1b = din("w1b", [36, 512])
    b1t = din("b1t", [128, 4], F32)
    w2t = din("w2t", [128, 4 * 256])
    b2t = din("b2t", [128, 2], F32)
    w3t = din("w3t", [128, 2 * 128])
    b3t = din("b3t", [128, 1], F32)
    out = nc.dram_tensor("out", [CO, P], F32, kind="ExternalOutput")

    with tile.TileContext(nc) as tc:
        with (
            tc.tile_pool(name="const", bufs=1) as cpool,
            tc.tile_pool(name="big", bufs=1) as bpool,
            tc.tile_pool(name="dram", bufs=1, space="DRAM") as dpool,
        ):
            # ---- resident inputs (mask-phase inputs FIRST so the PE
            # can start d2 matmuls while the big DMAs stream in) ------
            djlt = cpool.tile([5, N], BF16, name="djlt")
            nc.sync.dma_start(djlt[:], djl[:])
            drit = cpool.tile([5, P], BF16, name="drit")
            nc.sync.dma_start(drit[:], dri[:])
            gjt = cpool.tile([128, NB * 10], BF16, name="gjt")
            nc.sync.dma_start(gjt[:], gj[:])
            thrt = cpool.tile([128, 1], F32, name="thrt")
            nc.sync.dma_start(thrt[:], thrin[:])
            gfixt = cpool.tile([128, IC * 10], F32, name="gfixt")
            nc.sync.dma_start(gfixt[:], gfix[:])
            identt = cpool.tile([128, 128], F32, name="identt")
            nc.sync.dma_start(identt[:], identin[:])
            h2t = []
            for kb in range(4):
                t = bpool.tile([128, 2 * P], mybir.dt.float8e4,
                               name=f"h2t{kb}")
                nc.gpsimd.dma_start(t[:], h2s[kb * 128:(kb + 1) * 128, :])
                h2t.append(t)
            h1t = bpool.tile([128, P], BF16, name="h1t")
            nc.gpsimd.dma_start(h1t[:], h1s[:])

            def loadw(name, src, shape, dt=BF16):
                t = cpool.tile(shape, dt, name=name)
                nc.sync.dma_start(t[:], src[:])
                return t

            _consts = {}

            def constap(val):
                if val not in _consts:
                    t = cpool.tile([128, 1], F32,
                                   name=f"cst{len(_consts)}")
                    nc.gpsimd.memset(t[:], val)
                    _consts[val] = t
                return _consts[val][:]

            wdg1t = loadw("wdg1t", wdg1, [128, 4 * 2 * 256],
                          mybir.dt.float8e4)
            bdg1t = loadw("bdg1t", bdg1, [128, 2], F32)
            wdg2t = loadw("wdg2t", wdg2, [128, 2 * 64])
            bdg2t = loadw("bdg2t", bdg2, [64, 1], F32)
            wdg3t = loadw("wdg3t", wdg3, [64, 32])
            bdg3t = loadw("bdg3t", bdg3, [32, 1], F32)
            edw1t = loadw("edw1t", edw1, [3, 4])
            edb1t = loadw("edb1t", edb1, [4, 1], F32)
            edw2t = loadw("edw2t", edw2, [4, 4])
            edb2t = loadw("edb2t", edb2, [4, 1], F32)
            w1at = loadw("w1at", w1a, [128, 512])
            w1bt = loadw("w1bt", w1b, [36, 512])
            b1tt = loadw("b1tt", b1t, [128, 4], F32)
            w2tt = loadw("w2tt", w2t, [128, 4 * 256])
            b2tt = loadw("b2tt", b2t, [128, 2], F32)
            w3tt = loadw("w3tt", w3t, [128, 2 * 128])
            b3tt = loadw("b3tt", b3t, [128, 1], F32)

            # ---- warm-up: dense dummy matmuls so the PE HAM
            # un-throttles to 2.4 GHz before the real work, while the
            # input DMAs are still in flight --------------------------
            with tc.tile_pool(name="warmps", bufs=1, space="PSUM") as wp:
                wz = cpool.tile([128, 512], BF16, name="wz")
                nc.vector.memset(wz[:], 0.0)
                wps = wp.tile([128, 512], F32, name="wps")
                for _ in range(20):
                    nc.tensor.matmul(wps[:], wz[:, 0:128], wz[:],
                                     start=True, stop=True)

            # ---- phase B: mask + neighbor stats ----------------------
            # jb quads: {4k,4k+1} cmp on VectorE (is_lt, 0/1 mask),
            # {4k+2,4k+3} cmp on ScalarE (Sign form, weighted 0.5 with
            # the 0.5*sum(G) shift folded into gfix on the host).
            # S-matmuls for a same-kind jb pair share one PSUM
            # accumulation, halving the VectorE accumulate chain.
            s_acc = bpool.tile([128, IC * 10], F32, name="s_acc")
            with (
                tc.tile_pool(name="spsum", bufs=2, space="PSUM") as spool,
                tc.tile_pool(name="maskp", bufs=4) as mpool,
            ):
                with tc.tile_pool(name="d2psum", bufs=2, space="PSUM") as d2pool:
                    for pjb in range(NB // 2):
                        pair = (2 * pjb, 2 * pjb + 1)
                        is_sign = pair[0] in sign_jbs
                        masks = {}
                        for jb in pair:
                            assert (jb in sign_jbs) == is_sign
                            maskt = mpool.tile([128, P], BF16,
                                               name="maskt", tag="mask")
                            masks[jb] = maskt
                            for iw in range(NIW):
                                d2t = d2pool.tile([128, IW], F32,
                                                  name="d2t", tag="d2")
                                for c5 in range(IW // 512):
                                    lo = iw * IW + c5 * 512
                                    nc.tensor.matmul(
                                        d2t[:, c5 * 512:(c5 + 1) * 512],
                                        djlt[:, jb * 128:(jb + 1) * 128],
                                        drit[:, lo:lo + 512],
                                        start=True, stop=True,
                                    )
                                msl = maskt[:, iw * IW:(iw + 1) * IW]
                                if is_sign:
                                    nc.scalar.activation(
                                        msl, d2t[:], AF.Sign,
                                        bias=thrt[:], scale=-1.0,
                                    )
                                else:
                                    nc.vector.tensor_scalar(
                                        msl, d2t[:], thrt[:], None,
                                        ALU.is_lt)
                        sps = spool.tile([128, IC * 10], F32, name="sps",
                                         tag="sps")
                        for ic in range(IC):
                            for ji, jb in enumerate(pair):
                                nc.tensor.matmul(
                                    sps[:, ic * 10:(ic + 1) * 10],
                                    masks[jb][:, ic * 128:(ic + 1) * 128],
                                    gjt[:, jb * 10:(jb + 1) * 10],
                                    start=(ji == 0), stop=(ji == 1),
                                )
                        if pjb == 0:
                            nc.vector.tensor_copy(s_acc[:], sps[:])
                        elif is_sign:
                            nc.vector.scalar_tensor_tensor(
                                s_acc[:], sps[:], 0.5, s_acc[:],
                                op0=ALU.mult, op1=ALU.add)
                        else:
                            nc.vector.tensor_add(s_acc[:], s_acc[:], sps[:])

                # ---- phase C: S fixup into planes --------------------
                # planes layout [r, q*IC + c]
                planes = bpool.tile([128, 10 * IC], F32, name="planes")
                pl3 = planes.rearrange("p (q c) -> p q c", c=IC)
                for ic in range(IC):
                    ssl = slice(ic * 10, (ic + 1) * 10)
                    nc.vector.tensor_sub(
                        pl3[:, :, ic], s_acc[:, ssl], gfixt[:, ssl])

            # epilogue helper: relu(psum + bias) on alternating engines
            _epi_n = [0]

            def epilogue(dst, ps, bias_ap, relu=True):
                _epi_n[0] += 1
                if _epi_n[0] % 2 == 0:
                    nc.scalar.activation(
                        dst, ps, AF.Relu if relu else AF.Identity,
                        bias=bias_ap)
                elif relu:
                    nc.vector.tensor_scalar(
                        dst, ps, bias_ap, 0.0, ALU.add, ALU.max)
                else:
                    nc.vector.tensor_scalar(
                        dst, ps, bias_ap, None, ALU.add)

            # ---- phase F: DG convs 1024->256->64->32 (emitted BEFORE
            # the eigensolver chain so the PE has queued work) ---------
            with tc.tile_pool(name="cpsum2", bufs=4, space="PSUM") as pp:
                dg1 = [bpool.tile([128, P], BF16, name=f"dg1_{m}")
                       for m in range(2)]
                w1v = wdg1t.rearrange("p (kb i m) -> p kb i m",
                                      kb=4, i=2)
                h2v = [t.rearrange("p (i n) -> p i n", i=2) for t in h2t]
                for mo in range(2):
                    for pc in range(PC):
                        psl = slice(pc * PCW, (pc + 1) * PCW)
                        ps = pp.tile([128, PCW], F32, name="ps", tag="ps")
                        for kb in range(4):
                            nc.tensor.matmul(
                                ps[:],
                                w1v[:, kb, :,
                                    mo * 128:(mo + 1) * 128],
                                h2v[kb][:, :, psl],
                                start=(kb == 0), stop=(kb == 3),
                                perf_mode=mybir.MatmulPerfMode.DoubleRow,
                            )
                        epilogue(dg1[mo][:, psl], ps[:],
                                 bdg1t[:, mo:mo + 1])
                dg2 = bpool.tile([64, P], BF16, name="dg2")
                for pc in range(PC):
                    psl = slice(pc * PCW, (pc + 1) * PCW)
                    ps = pp.tile([64, PCW], F32, name="ps", tag="ps")
                    for kb in range(2):
                        nc.tensor.matmul(
                            ps[:], wdg2t[:, kb * 64:(kb + 1) * 64],
                            dg1[kb][:, psl],
                            start=(kb == 0), stop=(kb == 1))
                    epilogue(dg2[:, psl], ps[:], bdg2t[:])
                zcat = bpool.tile([36, P], BF16, name="zcat")
                for pc in range(PC):
                    psl = slice(pc * PCW, (pc + 1) * PCW)
                    ps = pp.tile([32, PCW], F32, name="ps", tag="ps")
                    nc.tensor.matmul(ps[:], wdg3t[:], dg2[:, psl],
                                     start=True, stop=True)
                    epilogue(zcat[0:32, psl], ps[:], bdg3t[:])

            # ---- phase D: closed-form 3x3 eigvalsh -------------------
            with tc.tile_pool(name="eig", bufs=1) as ep:
                def Pq(q):
                    return planes[:, q * IC:(q + 1) * IC]

                def T(name):
                    return ep.tile([128, IC], F32, name=name)

                v = nc.vector
                sc = nc.scalar
                n_, sx, sy, sz = Pq(0), Pq(1), Pq(2), Pq(3)
                sxx, sxy, sxz, syy, syz, szz = (Pq(i) for i in range(4, 10))
                ncl = T("ncl"); v.tensor_scalar_max(ncl[:], n_, 1.0)
                inv = T("inv"); v.reciprocal(inv[:], ncl[:])
                t0 = T("t0"); t1 = T("t1")
                cov = {}
                for nm, (a, b2_, s2ab) in dict(
                    xx=(sx, sx, sxx), xy=(sx, sy, sxy), xz=(sx, sz, sxz),
                    yy=(sy, sy, syy), yz=(sy, sz, syz), zz=(sz, sz, szz),
                ).items():
                    cab = T("c" + nm)
                    v.tensor_mul(t0[:], a, b2_)
                    v.tensor_mul(t0[:], t0[:], inv[:])
                    v.tensor_sub(cab[:], s2ab, t0[:])
                    cov[nm] = cab
                q_ = T("q_")
                v.tensor_add(q_[:], cov["xx"][:], cov["yy"][:])
                v.tensor_add(q_[:], q_[:], cov["zz"][:])
                v.tensor_scalar_mul(q_[:], q_[:], 1.0 / 3.0)
                for nm in ("xx", "yy", "zz"):
                    v.tensor_sub(cov[nm][:], cov[nm][:], q_[:])
                p2 = T("p2")
                v.tensor_mul(p2[:], cov["xx"][:], cov["xx"][:])
                v.tensor_mul(t0[:], cov["yy"][:], cov["yy"][:])
                v.tensor_add(p2[:], p2[:], t0[:])
                v.tensor_mul(t0[:], cov["zz"][:], cov["zz"][:])
                v.tensor_add(p2[:], p2[:], t0[:])
                v.tensor_mul(t1[:], cov["xy"][:], cov["xy"][:])
                v.tensor_mul(t0[:], cov["xz"][:], cov["xz"][:])
                v.tensor_add(t1[:], t1[:], t0[:])
                v.tensor_mul(t0[:], cov["yz"][:], cov["yz"][:])
                v.tensor_add(t1[:], t1[:], t0[:])
                v.scalar_tensor_tensor(
                    p2[:], t1[:], 2.0, p2[:], op0=ALU.mult, op1=ALU.add)
                v.tensor_scalar_max(p2[:], p2[:], 1e-30)
                pm = T("pm")
                sc.activation(pm[:], p2[:], AF.Sqrt, scale=1.0 / 6.0)
                ipm = T("ipm"); v.reciprocal(ipm[:], pm[:])
                for nm in cov:
                    v.tensor_mul(cov[nm][:], cov[nm][:], ipm[:])
                m1 = T("m1"); m2 = T("m2"); m3 = T("m3")
                v.tensor_mul(m1[:], cov["yy"][:], cov["zz"][:])
                v.tensor_mul(t0[:], cov["yz"][:], cov["yz"][:])
                v.tensor_sub(m1[:], m1[:], t0[:])
                v.tensor_mul(m2[:], cov["xy"][:], cov["zz"][:])
                v.tensor_mul(t0[:], cov["yz"][:], cov["xz"][:])
                v.tensor_sub(m2[:], m2[:], t0[:])
                v.tensor_mul(m3[:], cov["xy"][:], cov["yz"][:])
                v.tensor_mul(t0[:], cov["yy"][:], cov["xz"][:])
                v.tensor_sub(m3[:], m3[:], t0[:])
                det = T("det")
                v.tensor_mul(det[:], cov["xx"][:], m1[:])
                v.tensor_mul(t0[:], cov["xy"][:], m2[:])
                v.tensor_sub(det[:], det[:], t0[:])
                v.tensor_mul(t0[:], cov["xz"][:], m3[:])
                v.tensor_add(det[:], det[:], t0[:])
                r_ = T("r_")
                v.tensor_scalar(r_[:], det[:], 0.5, 1.0, ALU.mult, ALU.min)
                v.tensor_scalar_max(r_[:], r_[:], -1.0)
                # arccos(r) with arctan limited to [0, pi/4]
                ar = T("ar"); sc.activation(ar[:], r_[:], AF.Abs)
                sq = T("sq")
                v.tensor_mul(sq[:], r_[:], r_[:])
                v.tensor_scalar(sq[:], sq[:], -1.0, 1.0, ALU.mult, ALU.add)
                v.tensor_scalar_max(sq[:], sq[:], 0.0)
                sc.activation(sq[:], sq[:], AF.Sqrt)
                mn = T("mn"); mx = T("mx")
                v.tensor_tensor(mn[:], ar[:], sq[:], ALU.min)
                v.tensor_tensor(mx[:], ar[:], sq[:], ALU.max)
                imx = T("imx"); v.reciprocal(imx[:], mx[:])
                u_ = T("u_"); v.tensor_mul(u_[:], mn[:], imx[:])
                at = T("at"); sc.activation(at[:], u_[:], AF.Arctan)
                c1 = T("c1")
                v.tensor_tensor(c1[:], ar[:], sq[:], ALU.is_gt)
                mm_ = T("mm_")
                v.tensor_scalar(
                    mm_[:], at[:], 2.0, -math.pi / 2, ALU.mult, ALU.add)
                v.tensor_mul(mm_[:], mm_[:], c1[:])
                thp = T("thp")
                v.tensor_scalar(
                    thp[:], at[:], -1.0, math.pi / 2, ALU.mult, ALU.add)
                v.tensor_add(thp[:], thp[:], mm_[:])
                neg = T("neg")
                v.tensor_scalar(neg[:], r_[:], 0.0, None, ALU.is_lt)
                n2 = T("n2")
                v.tensor_scalar(
                    n2[:], thp[:], -2.0, math.pi, ALU.mult, ALU.add)
                v.tensor_mul(n2[:], n2[:], neg[:])
                th = T("th")
                v.tensor_add(th[:], thp[:], n2[:])
                # eigenvalues (ascending) into eigout [ls | lm | lb]
                eigout = bpool.tile([128, 3 * IC], F32, name="eigout")
                ls = eigout[:, 0:IC]
                lm = eigout[:, IC:2 * IC]
                lb = eigout[:, 2 * IC:3 * IC]
                cb = T("cb")
                sc.activation(cb[:], th[:], AF.Sin,
                              bias=constap(math.pi / 2), scale=1.0 / 3.0)
                ss = T("ss")
                sc.activation(ss[:], th[:], AF.Sin,
                              bias=constap(math.pi / 6), scale=1.0 / 3.0)
                v.tensor_mul(cb[:], cb[:], pm[:])
                v.tensor_mul(ss[:], ss[:], pm[:])
                v.scalar_tensor_tensor(
                    lb, cb[:], 2.0, q_[:], op0=ALU.mult, op1=ALU.add)
                v.scalar_tensor_tensor(
                    ls, ss[:], -2.0, q_[:], op0=ALU.mult, op1=ALU.add)
                v.tensor_scalar_mul(lm, q_[:], 3.0)
                v.tensor_sub(lm, lm, lb)
                v.tensor_sub(lm, lm, ls)

            # ---- phase E: transpose eig to channel-major + ED MLP ----
            with tc.tile_pool(name="cpsum", bufs=3, space="PSUM") as pp:
                trp = pp.tile([3 * IC, 128], F32, name="trp", tag="ps")
                nc.tensor.transpose(trp[:], eigout[:], identt[:])
                trsb = bpool.tile([3 * IC, 128], BF16, name="trsb")
                nc.vector.tensor_copy(trsb[:], trp[:])
                edram = dpool.tile([3 * IC, 128], BF16, name="edram")
                nc.sync.dma_start(edram[:], trsb[:])
                eigcm = bpool.tile([3, P], BF16, name="eigcm")
                nc.sync.dma_start(
                    eigcm[:], edram.rearrange("(q c) r -> q (c r)", q=3))

                ed1 = bpool.tile([4, P], BF16, name="ed1")
                for pc in range(PC):
                    psl = slice(pc * PCW, (pc + 1) * PCW)
                    e1p = pp.tile([4, PCW], F32, name="e1p", tag="ps")
                    nc.tensor.matmul(e1p[:], edw1t[:], eigcm[:, psl],
                                     start=True, stop=True)
                    nc.scalar.activation(ed1[:, psl], e1p[:], AF.Relu,
                                         bias=edb1t[:])
                for pc in range(PC):
                    psl = slice(pc * PCW, (pc + 1) * PCW)
                    e2p = pp.tile([4, PCW], F32, name="e2p", tag="ps")
                    nc.tensor.matmul(e2p[:], edw2t[:], ed1[:, psl],
                                     start=True, stop=True)
                    nc.scalar.activation(zcat[32:36, psl], e2p[:],
                                         AF.Identity, bias=edb2t[:])

                # ---- phase G: final convs 164->512->256->128 ---------
                z1 = [bpool.tile([128, P], BF16, name=f"z1_{m}")
                      for m in range(4)]
                for mo in range(4):
                    for pc in range(PC):
                        psl = slice(pc * PCW, (pc + 1) * PCW)
                        ps = pp.tile([128, PCW], F32, name="ps", tag="ps")
                        msl = slice(mo * 128, (mo + 1) * 128)
                        nc.tensor.matmul(ps[:], w1at[:, msl], h1t[:, psl],
                                         start=True, stop=False)
                        nc.tensor.matmul(ps[:], w1bt[:, msl], zcat[:, psl],
                                         start=False, stop=True)
                        epilogue(z1[mo][:, psl], ps[:],
                                 b1tt[:, mo:mo + 1])
                z2 = [bpool.tile([128, P], BF16, name=f"z2_{m}")
                      for m in range(2)]
                for mo in range(2):
                    for pc in range(PC):
                        psl = slice(pc * PCW, (pc + 1) * PCW)
                        ps = pp.tile([128, PCW], F32, name="ps", tag="ps")
                        for kb in range(4):
                            nc.tensor.matmul(
                                ps[:],
                                w2tt[:, kb * 256 + mo * 128:
                                     kb * 256 + (mo + 1) * 128],
                                z1[kb][:, psl],
                                start=(kb == 0), stop=(kb == 3))
                        epilogue(z2[mo][:, psl], ps[:],
                                 b2tt[:, mo:mo + 1])
                outz = bpool.tile([128, P], F32, name="outz")
                for pc in range(PC):
                    psl = slice(pc * PCW, (pc + 1) * PCW)
                    ps = pp.tile([128, PCW], F32, name="ps", tag="ps")
                    for kb in range(2):
                        nc.tensor.matmul(
                            ps[:], w3tt[:, kb * 128:(kb + 1) * 128],
                            z2[kb][:, psl],
                            start=(kb == 0), stop=(kb == 1))
                    epilogue(outz[:, psl], ps[:], b3tt[:])
                nc.sync.dma_start(out[:], outz[:])

    nc.compile()
    return nc


def _fib_directions(k=64):
    i = np.arange(k)
    phi = np.pi * (3.0 - np.sqrt(5.0)) * i
    ct = 1.0 - 2.0 * (i + 0.5) / k
    st = np.sqrt(np.maximum(1.0 - ct * ct, 0.0))
    u = np.stack([st * np.cos(phi), st * np.sin(phi), ct], -1)
    return np.concatenate([u, -u], 0)  # [2k, 3]


_DIRS = _fib_directions(64)


def host_prep(xyz, h1, h2_in, weights, cfg):
    """Build per-core in_maps. xyz/h1/h2_in are full f32 arrays."""
    N, P, NB, IC, PC, PCW, IW, NIW = _cfg_derived(cfg)
    sign_jbs = set(cfg.get("sign_jbs",
                       [j for j in range(NB) if j % 4 >= 2]))
    ncores = cfg.get("ncores", NCORES)
    nb_ = cfg.get("B", B)

    w = {k: np.asarray(v, np.float32) for k, v in weights.items()}
    # fold BN scale into weights, bias' = s*b + t
    def fold(wk, bk, sk, tk):
        W = (w[wk] * w[sk][:, None]).astype(np.float32)
        bias = (w[sk] * w[bk] + w[tk]).astype(np.float32)
        return W, bias

    Wdg1, bdg1 = fold("dg_w1", "dg_b1", "dg_s1", "dg_t1")
    Wdg2, bdg2 = fold("dg_w2", "dg_b2", "dg_s2", "dg_t2")
    Wdg3, bdg3 = fold("dg_w3", "dg_b3", "dg_s3", "dg_t3")
    W1, b1 = fold("w1", "b1", "s1", "t1")
    W2, b2 = fold("w2", "b2", "s2", "t2")
    W3, b3 = fold("w3", "b3", "s3", "t3")

    com = {}
    # DoubleRow fp8 layout: [p, kb, i, m], channel c = kb*256 + i*128 + p
    com["wdg1"] = (Wdg1.T.reshape(4, 2, 128, 256).transpose(2, 0, 1, 3)
                   .reshape(128, 4 * 2 * 256)
                   .astype(ml_dtypes.float8_e4m3))
    com["bdg1"] = bdg1.reshape(2, 128).T.copy()
    com["wdg2"] = (Wdg2.T.reshape(2, 128, 64).transpose(1, 0, 2)
                   .reshape(128, 128).astype(NP_BF16))
    com["bdg2"] = bdg2.reshape(64, 1).copy()
    com["wdg3"] = Wdg3.T.astype(NP_BF16)
    com["bdg3"] = bdg3.reshape(32, 1).copy()
    com["edw1"] = (w["ed_w1"].T / NPTS).astype(NP_BF16)
    com["edb1"] = w["ed_b1"].reshape(4, 1).copy()
    com["edw2"] = w["ed_w2"].T.astype(NP_BF16)
    com["edb2"] = w["ed_b2"].reshape(4, 1).copy()
    W1T = W1.T  # [164, 512]
    com["w1a"] = W1T[0:128].astype(NP_BF16)
    com["w1b"] = W1T[128:164].astype(NP_BF16)
    com["b1t"] = b1.reshape(4, 128).T.copy()
    com["w2t"] = (W2.T.reshape(4, 128, 256).transpose(1, 0, 2)
                  .reshape(128, 4 * 256).astype(NP_BF16))
    com["b2t"] = b2.reshape(2, 128).T.copy()
    com["w3t"] = (W3.T.reshape(2, 128, 128).transpose(1, 0, 2)
                  .reshape(128, 2 * 128).astype(NP_BF16))
    com["b3t"] = b3.reshape(1, 128).T.copy()
    com["identin"] = np.eye(128, dtype=np.float32)

    in_maps = []
    for c in range(ncores):
        bidx, h = c // 2, c % 2
        sl = slice(h * P, (h + 1) * P)
        X = np.asarray(xyz[bidx], np.float32)[:N]
        X = X - X.mean(0, keepdims=True)
        x2 = (X * X).sum(-1)
        one = np.ones_like(x2)
        djl = np.stack([X[:, 0], X[:, 1], X[:, 2], x2, one])
        Xi, x2i = X[sl], x2[sl]
        dri = np.stack([-2 * Xi[:, 0], -2 * Xi[:, 1], -2 * Xi[:, 2],
                        np.ones(P, np.float32), x2i])
        G = np.concatenate(
            [one[:, None], X,
             X[:, [0, 0, 0, 1, 1, 2]] * X[:, [0, 1, 2, 1, 2, 2]]], 1)
        # [N, 10]: 1, x, y, z, xx, xy, xz, yy, yz, zz
        gj = (G.reshape(NB, 128, 10).transpose(1, 0, 2)
              .reshape(128, NB * 10))
        gsgn = np.zeros(10, np.float32)
        for jb in sign_jbs:
            gsgn += G[jb * 128:(jb + 1) * 128].sum(0)
        gfix = G[sl] - 0.5 * gsgn  # [P, 10]
        gfixt = (gfix.reshape(IC, 128, 10).transpose(1, 0, 2)
                 .reshape(128, IC * 10))
        proj = X @ _DIRS.T.astype(np.float32)  # [N, 128]
        pmax = proj.max(0)
        diam = float((pmax[:64] + pmax[64:]).max())
        thr = np.full((128, 1), (FACTOR * diam) ** 2, np.float32)

        m = dict(com)
        h2sl = np.asarray(h2_in[bidx], np.float32)[:, sl]  # [1024, P]
        m["h2s"] = np.ascontiguousarray(
            h2sl.reshape(4, 2, 128, P).transpose(0, 2, 1, 3)
            .reshape(4 * 128, 2 * P)).astype(ml_dtypes.float8_e4m3)
        m["h1s"] = np.ascontiguousarray(
            np.asarray(h1[bidx], np.float32)[:, sl]).astype(NP_BF16)
        m["djl"] = djl.astype(NP_BF16)
        m["dri"] = dri.astype(NP_BF16)
        m["gj"] = gj.astype(NP_BF16)
        m["gfix"] = gfixt.astype(np.float32)
        m["thrin"] = thr
        in_maps.append(m)
    return in_maps


_NC_CACHE = {}


def _get_nc(cfg_key=None):
    if "nc" not in _NC_CACHE:
        _NC_CACHE["nc"] = build_nc(dict(FULL_CFG))
    return _NC_CACHE["nc"]


def kernel(**inputs):
    from concourse.bass_utils import run_bass_kernel_spmd

    xyz = np.asarray(inputs["xyz"], np.float32)
    h1 = np.asarray(inputs["h1"], np.float32)
    h2_in = np.asarray(inputs["h2_in"], np.float32)
    weights = {k: v for k, v in inputs.items()
               if k not in ("xyz", "h1", "h2_in")}

    nc = _get_nc()
    cfg = dict(FULL_CFG)
    in_maps = host_prep(xyz, h1, h2_in, weights, cfg)
    res = run_bass_kernel_spmd(nc, in_maps, core_ids=list(range(NCORES)))
    P = cfg["P"]
    z = np.empty((B, CO, NPTS), np.float32)
    for c in range(NCORES):
        bidx, h = c // 2, c % 2
        z[bidx, :, h * P:(h + 1) * P] = res.results[c]["out"]
    return (inputs["xyz"], z)


# revision 19
# speedup vs baseline: 1.4464x; 1.1132x over previous
"""Trainium2 Bass kernel for nn_Adaptive_EDDG (gnn_message_passing).

Sharding: 8 cores = 4 batches x 2 point-halves (pure data-parallel SPMD,
no collectives).  Each core owns P=2048 points (i) of one batch and the
full N=4096 neighbor set (j).

Device pipeline per core:
  - d2 tiles [128 j, P i] via K=5 bf16 matmul:  [x,y,z,|x|^2,1]_j^T @
    [-2x,-2y,-2z,1,|x|^2]_i  -> pairwise squared distances in PSUM.
  - threshold (d2 < radius^2) into a 0/1 bf16 mask; split across
    VectorE (is_lt) and ScalarE (Sign trick, fixed up linearly).
  - mask tiles used as matmul weights vs per-point moments G[j,0:10] =
    [1,x,y,z,xx,xy,xz,yy,yz,zz] -> neighbor stats cnt/S1/S2 per point.
  - closed-form symmetric 3x3 eigvalsh (trig method, range-safe arccos
    via arctan) on [128,16] point-major planes.
  - ED 3->4->4 MLP, DG 1024->256->64->32 convs, concat with h1,
    164->512->256->128 convs; BN folded into weights on host; bf16
    matmuls with fp32 accumulation.

The radius is max pairwise distance * 0.1; computed on host via a
128-direction projection diameter (>=98% exact; final output is
insensitive to radius at the 1e-5 level, measured).
"""

import math
import os
import sys

import numpy as np

try:
    import concourse.bacc as bacc  # noqa: F401
except Exception:  # pragma: no cover
    for _p in ("/opt/trn_rl_repo", "/root/.axon_site/_ro/trn_rl_repo"):
        if os.path.isdir(_p) and _p not in sys.path:
            sys.path.insert(0, _p)
    import concourse.bacc as bacc

import ml_dtypes
import concourse.bass as bass
import concourse.mybir as mybir
import concourse.tile as tile

BF16 = mybir.dt.bfloat16
F32 = mybir.dt.float32
AF = mybir.ActivationFunctionType
ALU = mybir.AluOpType
NP_BF16 = ml_dtypes.bfloat16

B, NPTS, SA, DGD, EDC, CO = 4, 4096, 128, 1024, 4, 128
FACTOR = 0.1
NCORES = 8

FULL_CFG = dict(N=NPTS, P=NPTS // 2)


def _cfg_derived(cfg):
    N, P = cfg["N"], cfg["P"]
    NB = N // 128       # j blocks
    IC = P // 128       # i chunks
    PC = max(P // 512, 1)  # point chunks for convs
    PCW = min(P, 512)
    IW = min(P, 1024)   # d2 psum tile width
    NIW = P // IW
    return N, P, NB, IC, PC, PCW, IW, NIW


def _default_sign_jbs(NB):
    """jb pairs assigned to the ScalarE Sign-form compare."""
    out = []
    for p in range(NB // 2):
        if (NB >= 16 and p % 8 >= 3) or (NB < 16 and p % 2 == 1):
            out.extend([2 * p, 2 * p + 1])
    return out


def build_nc(cfg):
    """Build the SPMD single-core program (same graph on all 8 cores)."""
    N, P, NB, IC, PC, PCW, IW, NIW = _cfg_derived(cfg)
    sign_jbs = set(cfg.get("sign_jbs", _default_sign_jbs(NB)))
    HP = P // 2          # points per i-half
    ICH = HP // 128      # i-chunks per half
    CW = min(HP, 512)    # conv chunk width
    NCW = HP // CW       # conv chunks per half
    DW = min(HP, 1024)   # d2 psum tile width
    NDW = HP // DW

    nc = bacc.Bacc("TRN2", target_bir_lowering=False, debug=False)

    def din(name, shape, dt=BF16):
        return nc.dram_tensor(name, shape, dt, kind="ExternalInput")

    h2s = din("h2s", [DGD // 2, 2 * P], mybir.dt.float8e4)
    h1s = din("h1s", [SA, P])
    djl = din("djl", [128, N])     # rows 0-4 = x,y,z,|x|^2,1; rest 0
    dri = din("dri", [128, P])     # rows 0-4 = -2x,-2y,-2z,1,|x|^2; rest 0
    gj = din("gj", [128, NB * 10])
    gfix = din("gfix", [128, IC * 10], F32)
    thrin = din("thrin", [128, 1], F32)
    identin = din("identin", [128, 128], F32)
    wdg1 = din("wdg1", [128, 4 * 2 * 256], mybir.dt.float8e4)
    bdg1 = din("bdg1", [128, 2], F32)
    wdg2 = din("wdg2", [128, 2 * 64])
    bdg2 = din("bdg2", [64, 1], F32)
    wdg3 = din("wdg3", [64, 32])
    bdg3 = din("bdg3", [32, 1], F32)
    edw1 = din("edw1", [3, 4])
    edb1 = din("edb1", [4, 1], F32)
    edw2 = din("edw2", [4, 4])
    edb2 = din("edb2", [4, 1], F32)
    w1a = din("w1a", [128, 512])
    w1b = din("w1b", [128, 512])   # rows 0-35 = h2|h3 block; rest 0
    b1t = din("b1t", [128, 4], F32)
    w2t = din("w2t", [128, 4 * 256])
    b2t = din("b2t", [128, 2], F32)
    w3t = din("w3t", [128, 2 * 128])
    b3t = din("b3t", [128, 1], F32)
    out = nc.dram_tensor("out", [CO, P], F32, kind="ExternalOutput")

    with tile.TileContext(nc) as tc:
        with (
            tc.tile_pool(name="const", bufs=1) as cpool,
            tc.tile_pool(name="big", bufs=1) as bpool,
            tc.tile_pool(name="dram", bufs=1, space="DRAM") as dpool,
            tc.tile_pool(name="spsum", bufs=2, space="PSUM") as spool,
            tc.tile_pool(name="d2psum", bufs=2, space="PSUM") as d2pool,
            tc.tile_pool(name="cpsum", bufs=2, space="PSUM") as pp,
            tc.tile_pool(name="maskp", bufs=4) as mpool,
        ):
            # ---- resident inputs (mask-phase inputs FIRST so the PE
            # can start d2 matmuls while the big DMAs stream in) ------
            djlt = cpool.tile([128, N], BF16, name="djlt")
            nc.sync.dma_start(djlt[:], djl[:])
            drit = cpool.tile([128, P], BF16, name="drit")
            nc.sync.dma_start(drit[:], dri[:])
            gjt = cpool.tile([128, NB * 10], BF16, name="gjt")
            nc.sync.dma_start(gjt[:], gj[:])
            thrt = cpool.tile([128, 1], F32, name="thrt")
            nc.sync.dma_start(thrt[:], thrin[:])
            gfixt = cpool.tile([128, IC * 10], F32, name="gfixt")
            nc.sync.dma_start(gfixt[:], gfix[:])
            identt = cpool.tile([128, 128], F32, name="identt")
            nc.sync.dma_start(identt[:], identin[:])
            h2t = []
            for kb in range(4):
                t = bpool.tile([128, 2 * P], mybir.dt.float8e4,
                               name=f"h2t{kb}")
                nc.gpsimd.dma_start(t[:], h2s[kb * 128:(kb + 1) * 128, :])
                h2t.append(t)
            h1t = bpool.tile([128, P], BF16, name="h1t")
            nc.gpsimd.dma_start(h1t[:], h1s[:])

            def loadw(name, src, shape, dt=BF16):
                t = cpool.tile(shape, dt, name=name)
                nc.sync.dma_start(t[:], src[:])
                return t

            _consts = {}

            def constap(val):
                if val not in _consts:
                    t = cpool.tile([128, 1], F32,
                                   name=f"cst{len(_consts)}")
                    nc.gpsimd.memset(t[:], val)
                    _consts[val] = t
                return _consts[val][:]

            wdg1t = loadw("wdg1t", wdg1, [128, 4 * 2 * 256],
                          mybir.dt.float8e4)
            bdg1t = loadw("bdg1t", bdg1, [128, 2], F32)
            wdg2t = loadw("wdg2t", wdg2, [128, 2 * 64])
            bdg2t = loadw("bdg2t", bdg2, [64, 1], F32)
            wdg3t = loadw("wdg3t", wdg3, [64, 32])
            bdg3t = loadw("bdg3t", bdg3, [32, 1], F32)
            edw1t = loadw("edw1t", edw1, [3, 4])
            edb1t = loadw("edb1t", edb1, [4, 1], F32)
            edw2t = loadw("edw2t", edw2, [4, 4])
            edb2t = loadw("edb2t", edb2, [4, 1], F32)
            w1at = loadw("w1at", w1a, [128, 512])
            w1bt = loadw("w1bt", w1b, [128, 512])
            b1tt = loadw("b1tt", b1t, [128, 4], F32)
            w2tt = loadw("w2tt", w2t, [128, 4 * 256])
            b2tt = loadw("b2tt", b2t, [128, 2], F32)
            w3tt = loadw("w3tt", w3t, [128, 2 * 128])
            b3tt = loadw("b3tt", b3t, [128, 1], F32)

            # zcat: rows 0-31 DG out, 32-35 ED out, 36-127 zero-pad so
            # the final conv K-block is a full 128 (fast weight load).
            zcat = bpool.tile([128, P], BF16, name="zcat")
            for zp in (32, 64, 96):
                nc.vector.memset(zcat[zp:zp + 32, :], 0.0)

            # ---- warm-up: dense dummy matmuls so the PE HAM
            # un-throttles to 2.4 GHz while input DMAs stream in ------
            wz = cpool.tile([128, 512], BF16, name="wz")
            nc.vector.memset(wz[:], 0.0)
            wps = pp.tile([128, 512], F32, name="wps", tag="ps")
            for _ in range(20):
                nc.tensor.matmul(wps[:], wz[:, 0:128], wz[:],
                                 start=True, stop=True)

            # ---- phase B: mask + neighbor stats, per i-half ----------
            # jb quads: {4k,4k+1} cmp on VectorE (is_lt, 0/1 mask),
            # {4k+2,4k+3} on ScalarE (Sign form, weighted 0.5 with the
            # 0.5*sum(G) shift folded into gfix on the host).  The
            # i-space is processed in two halves so the half-A
            # eigensolver chain overlaps half-B mask work on the PE.
            s_acc = [bpool.tile([128, ICH * 10], F32, name=f"s_acc{h}")
                     for h in range(2)]
            planes = [bpool.tile([128, 10 * ICH], F32, name=f"planes{h}")
                      for h in range(2)]
            eigout = [bpool.tile([128, 3 * ICH], F32, name=f"eigout{h}")
                      for h in range(2)]
            eigcm = bpool.tile([3, P], BF16, name="eigcm")
            ed1 = bpool.tile([4, P], BF16, name="ed1")

            def mask_half(h):
                plo = h * HP
                for pjb in range(NB // 2):
                    pair = (2 * pjb, 2 * pjb + 1)
                    is_sign = pair[0] in sign_jbs
                    masks = {}
                    for jb in pair:
                        assert (jb in sign_jbs) == is_sign
                        maskt = mpool.tile([128, HP], BF16,
                                           name="maskt", tag="mask")
                        masks[jb] = maskt
                        for iw in range(NDW):
                            d2t = d2pool.tile([128, DW], F32,
                                              name="d2t", tag="d2")
                            w5 = min(DW, 512)
                            for c5 in range(DW // w5):
                                lo = plo + iw * DW + c5 * w5
                                nc.tensor.matmul(
                                    d2t[:, c5 * w5:(c5 + 1) * w5],
                                    djlt[:, jb * 128:(jb + 1) * 128],
                                    drit[:, lo:lo + w5],
                                    start=True, stop=True,
                                )
                            msl = maskt[:, iw * DW:(iw + 1) * DW]
                            if is_sign:
                                nc.scalar.activation(
                                    msl, d2t[:], AF.Sign,
                                    bias=thrt[:], scale=-1.0)
                            else:
                                nc.vector.tensor_scalar(
                                    msl, d2t[:], thrt[:], None, ALU.is_lt)
                    sps = spool.tile([128, ICH * 10], F32, name="sps",
                                     tag="sps")
                    for ic in range(ICH):
                        for ji, jb in enumerate(pair):
                            nc.tensor.matmul(
                                sps[:, ic * 10:(ic + 1) * 10],
                                masks[jb][:, ic * 128:(ic + 1) * 128],
                                gjt[:, jb * 10:(jb + 1) * 10],
                                start=(ji == 0), stop=(ji == 1),
                            )
                    if pjb == 0:
                        nc.vector.tensor_copy(s_acc[h][:], sps[:])
                    elif is_sign:
                        nc.vector.scalar_tensor_tensor(
                            s_acc[h][:], sps[:], 0.5, s_acc[h][:],
                            op0=ALU.mult, op1=ALU.add)
                    else:
                        nc.vector.tensor_add(
                            s_acc[h][:], s_acc[h][:], sps[:])

            def epilogue(dst, ps, bias_ap, relu=True):
                nc.scalar.activation(
                    dst, ps, AF.Relu if relu else AF.Identity,
                    bias=bias_ap)

            def eig_half(h):
                """Planes fixup + closed-form 3x3 eigvalsh for half h."""
                for ic in range(ICH):
                    ssl = slice(ic * 10, (ic + 1) * 10)
                    gsl = slice((h * ICH + ic) * 10,
                                (h * ICH + ic + 1) * 10)
                    nc.vector.tensor_sub(
                        planes[h].rearrange(
                            "p (q c) -> p q c", c=ICH)[:, :, ic],
                        s_acc[h][:, ssl], gfixt[:, gsl])
                pl = planes[h]

                def Pq(q):
                    return pl[:, q * ICH:(q + 1) * ICH]

                with tc.tile_pool(name=f"eig{h}", bufs=1) as ep:
                    def T(name):
                        return ep.tile([128, ICH], F32,
                                       name=f"{name}_{h}")

                    v = nc.vector
                    sc = nc.scalar
                    n_, sx, sy, sz = Pq(0), Pq(1), Pq(2), Pq(3)
                    sxx, sxy, sxz, syy, syz, szz = (
                        Pq(i) for i in range(4, 10))
                    ncl = T("ncl"); v.tensor_scalar_max(ncl[:], n_, 1.0)
                    inv = T("inv"); v.reciprocal(inv[:], ncl[:])
                    t0 = T("t0"); t1 = T("t1")
                    cov = {}
                    for nm, (a, b2_, s2ab) in dict(
                        xx=(sx, sx, sxx), xy=(sx, sy, sxy),
                        xz=(sx, sz, sxz), yy=(sy, sy, syy),
                        yz=(sy, sz, syz), zz=(sz, sz, szz),
                    ).items():
                        cab = T("c" + nm)
                        v.tensor_mul(t0[:], a, b2_)
                        v.tensor_mul(t0[:], t0[:], inv[:])
                        v.tensor_sub(cab[:], s2ab, t0[:])
                        cov[nm] = cab
                    q_ = T("q_")
                    v.tensor_add(q_[:], cov["xx"][:], cov["yy"][:])
                    v.tensor_add(q_[:], q_[:], cov["zz"][:])
                    v.tensor_scalar_mul(q_[:], q_[:], 1.0 / 3.0)
                    for nm in ("xx", "yy", "zz"):
                        v.tensor_sub(cov[nm][:], cov[nm][:], q_[:])
                    p2 = T("p2")
                    v.tensor_mul(p2[:], cov["xx"][:], cov["xx"][:])
                    v.tensor_mul(t0[:], cov["yy"][:], cov["yy"][:])
                    v.tensor_add(p2[:], p2[:], t0[:])
                    v.tensor_mul(t0[:], cov["zz"][:], cov["zz"][:])
                    v.tensor_add(p2[:], p2[:], t0[:])
                    v.tensor_mul(t1[:], cov["xy"][:], cov["xy"][:])
                    v.tensor_mul(t0[:], cov["xz"][:], cov["xz"][:])
                    v.tensor_add(t1[:], t1[:], t0[:])
                    v.tensor_mul(t0[:], cov["yz"][:], cov["yz"][:])
                    v.tensor_add(t1[:], t1[:], t0[:])
                    v.scalar_tensor_tensor(
                        p2[:], t1[:], 2.0, p2[:],
                        op0=ALU.mult, op1=ALU.add)
                    v.tensor_scalar_max(p2[:], p2[:], 1e-30)
                    pm = T("pm")
                    sc.activation(pm[:], p2[:], AF.Sqrt, scale=1.0 / 6.0)
                    ipm = T("ipm"); v.reciprocal(ipm[:], pm[:])
                    for nm in cov:
                        v.tensor_mul(cov[nm][:], cov[nm][:], ipm[:])
                    m1 = T("m1"); m2 = T("m2"); m3 = T("m3")
                    v.tensor_mul(m1[:], cov["yy"][:], cov["zz"][:])
                    v.tensor_mul(t0[:], cov["yz"][:], cov["yz"][:])
                    v.tensor_sub(m1[:], m1[:], t0[:])
                    v.tensor_mul(m2[:], cov["xy"][:], cov["zz"][:])
                    v.tensor_mul(t0[:], cov["yz"][:], cov["xz"][:])
                    v.tensor_sub(m2[:], m2[:], t0[:])
                    v.tensor_mul(m3[:], cov["xy"][:], cov["yz"][:])
                    v.tensor_mul(t0[:], cov["yy"][:], cov["xz"][:])
                    v.tensor_sub(m3[:], m3[:], t0[:])
                    det = T("det")
                    v.tensor_mul(det[:], cov["xx"][:], m1[:])
                    v.tensor_mul(t0[:], cov["xy"][:], m2[:])
                    v.tensor_sub(det[:], det[:], t0[:])
                    v.tensor_mul(t0[:], cov["xz"][:], m3[:])
                    v.tensor_add(det[:], det[:], t0[:])
                    r_ = T("r_")
                    v.tensor_scalar(r_[:], det[:], 0.5, 1.0,
                                    ALU.mult, ALU.min)
                    v.tensor_scalar_max(r_[:], r_[:], -1.0)
                    # arccos(r) with arctan limited to [0, pi/4]
                    ar = T("ar"); sc.activation(ar[:], r_[:], AF.Abs)
                    sq = T("sq")
                    v.tensor_mul(sq[:], r_[:], r_[:])
                    v.tensor_scalar(sq[:], sq[:], -1.0, 1.0,
                                    ALU.mult, ALU.add)
                    v.tensor_scalar_max(sq[:], sq[:], 0.0)
                    sc.activation(sq[:], sq[:], AF.Sqrt)
                    mn = T("mn"); mx = T("mx")
                    v.tensor_tensor(mn[:], ar[:], sq[:], ALU.min)
                    v.tensor_tensor(mx[:], ar[:], sq[:], ALU.max)
                    imx = T("imx"); v.reciprocal(imx[:], mx[:])
                    u_ = T("u_"); v.tensor_mul(u_[:], mn[:], imx[:])
                    at = T("at"); sc.activation(at[:], u_[:], AF.Arctan)
                    c1 = T("c1")
                    v.tensor_tensor(c1[:], ar[:], sq[:], ALU.is_gt)
                    mm_ = T("mm_")
                    v.tensor_scalar(mm_[:], at[:], 2.0, -math.pi / 2,
                                    ALU.mult, ALU.add)
                    v.tensor_mul(mm_[:], mm_[:], c1[:])
                    thp = T("thp")
                    v.tensor_scalar(thp[:], at[:], -1.0, math.pi / 2,
                                    ALU.mult, ALU.add)
                    v.tensor_add(thp[:], thp[:], mm_[:])
                    neg = T("neg")
                    v.tensor_scalar(neg[:], r_[:], 0.0, None, ALU.is_lt)
                    n2 = T("n2")
                    v.tensor_scalar(n2[:], thp[:], -2.0, math.pi,
                                    ALU.mult, ALU.add)
                    v.tensor_mul(n2[:], n2[:], neg[:])
                    th = T("th")
                    v.tensor_add(th[:], thp[:], n2[:])
                    eo = eigout[h]
                    ls = eo[:, 0:ICH]
                    lm = eo[:, ICH:2 * ICH]
                    lb = eo[:, 2 * ICH:3 * ICH]
                    cb = T("cb")
                    sc.activation(cb[:], th[:], AF.Sin,
                                  bias=constap(math.pi / 2),
                                  scale=1.0 / 3.0)
                    ss = T("ss")
                    sc.activation(ss[:], th[:], AF.Sin,
                                  bias=constap(math.pi / 6),
                                  scale=1.0 / 3.0)
                    v.tensor_mul(cb[:], cb[:], pm[:])
                    v.tensor_mul(ss[:], ss[:], pm[:])
                    v.scalar_tensor_tensor(
                        lb, cb[:], 2.0, q_[:], op0=ALU.mult, op1=ALU.add)
                    v.scalar_tensor_tensor(
                        ls, ss[:], -2.0, q_[:], op0=ALU.mult, op1=ALU.add)
                    v.tensor_scalar_mul(lm, q_[:], 3.0)
                    v.tensor_sub(lm, lm, lb)
                    v.tensor_sub(lm, lm, ls)

            def ed_half(h):
                """Transpose eig -> channel-major + ED MLP for half h."""
                plo = h * HP
                trp = pp.tile([3 * ICH, 128], F32, name="trp", tag="ps")
                nc.tensor.transpose(trp[:], eigout[h][:], identt[:])
                trsb = bpool.tile([3 * ICH, 128], BF16, name="trsb",
                                  tag="trsb")
                nc.vector.tensor_copy(trsb[:], trp[:])
                edram = dpool.tile([3 * ICH, 128], BF16,
                                   name=f"edram{h}")
                nc.sync.dma_start(edram[:], trsb[:])
                nc.sync.dma_start(
                    eigcm[:, plo:plo + HP],
                    edram.rearrange("(q c) r -> q (c r)", q=3))
                for cc in range(NCW):
                    psl = slice(plo + cc * CW, plo + (cc + 1) * CW)
                    e1p = pp.tile([4, CW], F32, name="e1p", tag="ps")
                    nc.tensor.matmul(e1p[:], edw1t[:], eigcm[:, psl],
                                     start=True, stop=True)
                    nc.scalar.activation(ed1[:, psl], e1p[:], AF.Relu,
                                         bias=edb1t[:])
                for cc in range(NCW):
                    psl = slice(plo + cc * CW, plo + (cc + 1) * CW)
                    e2p = pp.tile([4, CW], F32, name="e2p", tag="ps")
                    nc.tensor.matmul(e2p[:], edw2t[:], ed1[:, psl],
                                     start=True, stop=True)
                    nc.scalar.activation(zcat[32:36, psl], e2p[:],
                                         AF.Identity, bias=edb2t[:])

            # ---- DG convs 1024->256->64->32 (independent of eig) -----
            dg1 = [bpool.tile([128, P], BF16, name=f"dg1_{m}")
                   for m in range(2)]
            dg2 = bpool.tile([64, P], BF16, name="dg2")
            w1v = wdg1t.rearrange("p (kb i m) -> p kb i m", kb=4, i=2)
            h2v = [t.rearrange("p (i n) -> p i n", i=2) for t in h2t]

            def dg_convs(pcs):
                for mo in range(2):
                    for pc in pcs:
                        psl = slice(pc * PCW, (pc + 1) * PCW)
                        ps = pp.tile([128, PCW], F32, name="ps", tag="ps")
                        for kb in range(4):
                            nc.tensor.matmul(
                                ps[:],
                                w1v[:, kb, :, mo * 128:(mo + 1) * 128],
                                h2v[kb][:, :, psl],
                                start=(kb == 0), stop=(kb == 3),
                                perf_mode=mybir.MatmulPerfMode.DoubleRow,
                            )
                        epilogue(dg1[mo][:, psl], ps[:],
                                 bdg1t[:, mo:mo + 1])
                for pc in pcs:
                    psl = slice(pc * PCW, (pc + 1) * PCW)
                    ps = pp.tile([64, PCW], F32, name="ps", tag="ps")
                    for kb in range(2):
                        nc.tensor.matmul(
                            ps[:], wdg2t[:, kb * 64:(kb + 1) * 64],
                            dg1[kb][:, psl],
                            start=(kb == 0), stop=(kb == 1))
                    epilogue(dg2[:, psl], ps[:], bdg2t[:])
                for pc in pcs:
                    psl = slice(pc * PCW, (pc + 1) * PCW)
                    ps = pp.tile([32, PCW], F32, name="ps", tag="ps")
                    nc.tensor.matmul(ps[:], wdg3t[:], dg2[:, psl],
                                     start=True, stop=True)
                    epilogue(zcat[0:32, psl], ps[:], bdg3t[:])

            # ---- final convs 164(pad 256)->512->256->128 per half ----
            z1 = [bpool.tile([128, P], BF16, name=f"z1_{m}")
                  for m in range(4)]
            z2 = [bpool.tile([128, P], BF16, name=f"z2_{m}")
                  for m in range(2)]
            outz = bpool.tile([128, P], F32, name="outz")

            def z_convs(h):
                plo = h * HP
                for cc in range(NCW):
                    psl = slice(plo + cc * CW, plo + (cc + 1) * CW)
                    for mo in range(4):
                        ps = pp.tile([128, CW], F32, name="ps", tag="ps")
                        msl = slice(mo * 128, (mo + 1) * 128)
                        nc.tensor.matmul(ps[:], w1at[:, msl],
                                         h1t[:, psl],
                                         start=True, stop=False)
                        nc.tensor.matmul(ps[:], w1bt[:, msl],
                                         zcat[:, psl],
                                         start=False, stop=True)
                        epilogue(z1[mo][:, psl], ps[:],
                                 b1tt[:, mo:mo + 1])
                    for mo in range(2):
                        ps = pp.tile([128, CW], F32, name="ps", tag="ps")
                        for kb in range(4):
                            nc.tensor.matmul(
                                ps[:],
                                w2tt[:, kb * 256 + mo * 128:
                                     kb * 256 + (mo + 1) * 128],
                                z1[kb][:, psl],
                                start=(kb == 0), stop=(kb == 3))
                        epilogue(z2[mo][:, psl], ps[:],
                                 b2tt[:, mo:mo + 1])
                    ps = pp.tile([128, CW], F32, name="ps", tag="ps")
                    for kb in range(2):
                        nc.tensor.matmul(
                            ps[:], w3tt[:, kb * 128:(kb + 1) * 128],
                            z2[kb][:, psl],
                            start=(kb == 0), stop=(kb == 1))
                    epilogue(outz[:, psl], ps[:], b3tt[:])
                nc.sync.dma_start(out[:, plo:plo + HP],
                                  outz[:, plo:plo + HP])

            # ---- emission order engineered for overlap ---------------
            mask_half(0)
            mask_half(1)
            eig_half(0)           # DVE chain; PE runs DG convs below
            dg_convs(range(PC))
            ed_half(0)
            eig_half(1)           # DVE chain; PE runs half-A z convs
            z_convs(0)
            ed_half(1)
            z_convs(1)

    nc.compile()
    return nc


def _fib_directions(k=64):
    i = np.arange(k)
    phi = np.pi * (3.0 - np.sqrt(5.0)) * i
    ct = 1.0 - 2.0 * (i + 0.5) / k
    st = np.sqrt(np.maximum(1.0 - ct * ct, 0.0))
    u = np.stack([st * np.cos(phi), st * np.sin(phi), ct], -1)
    return np.concatenate([u, -u], 0)  # [2k, 3]


_DIRS = _fib_directions(64)


def host_prep(xyz, h1, h2_in, weights, cfg):
    """Build per-core in_maps. xyz/h1/h2_in are full f32 arrays."""
    N, P, NB, IC, PC, PCW, IW, NIW = _cfg_derived(cfg)
    sign_jbs = set(cfg.get("sign_jbs",
                       [j for j in range(NB) if j % 4 >= 2]))
    ncores = cfg.get("ncores", NCORES)
    nb_ = cfg.get("B", B)

    w = {k: np.asarray(v, np.float32) for k, v in weights.items()}
    # fold BN scale into weights, bias' = s*b + t
    def fold(wk, bk, sk, tk):
        W = (w[wk] * w[sk][:, None]).astype(np.float32)
        bias = (w[sk] * w[bk] + w[tk]).astype(np.float32)
        return W, bias

    Wdg1, bdg1 = fold("dg_w1", "dg_b1", "dg_s1", "dg_t1")
    Wdg2, bdg2 = fold("dg_w2", "dg_b2", "dg_s2", "dg_t2")
    Wdg3, bdg3 = fold("dg_w3", "dg_b3", "dg_s3", "dg_t3")
    W1, b1 = fold("w1", "b1", "s1", "t1")
    W2, b2 = fold("w2", "b2", "s2", "t2")
    W3, b3 = fold("w3", "b3", "s3", "t3")

    com = {}
    # DoubleRow fp8 layout: [p, kb, i, m], channel c = kb*256 + i*128 + p
    com["wdg1"] = (Wdg1.T.reshape(4, 2, 128, 256).transpose(2, 0, 1, 3)
                   .reshape(128, 4 * 2 * 256)
                   .astype(ml_dtypes.float8_e4m3))
    com["bdg1"] = bdg1.reshape(2, 128).T.copy()
    com["wdg2"] = (Wdg2.T.reshape(2, 128, 64).transpose(1, 0, 2)
                   .reshape(128, 128).astype(NP_BF16))
    com["bdg2"] = bdg2.reshape(64, 1).copy()
    com["wdg3"] = Wdg3.T.astype(NP_BF16)
    com["bdg3"] = bdg3.reshape(32, 1).copy()
    com["edw1"] = (w["ed_w1"].T / NPTS).astype(NP_BF16)
    com["edb1"] = w["ed_b1"].reshape(4, 1).copy()
    com["edw2"] = w["ed_w2"].T.astype(NP_BF16)
    com["edb2"] = w["ed_b2"].reshape(4, 1).copy()
    W1T = W1.T  # [164, 512]
    com["w1a"] = W1T[0:128].astype(NP_BF16)
    w1bp = np.zeros((128, 512), np.float32)
    w1bp[0:36] = W1T[128:164]
    com["w1b"] = w1bp.astype(NP_BF16)
    com["b1t"] = b1.reshape(4, 128).T.copy()
    com["w2t"] = (W2.T.reshape(4, 128, 256).transpose(1, 0, 2)
                  .reshape(128, 4 * 256).astype(NP_BF16))
    com["b2t"] = b2.reshape(2, 128).T.copy()
    com["w3t"] = (W3.T.reshape(2, 128, 128).transpose(1, 0, 2)
                  .reshape(128, 2 * 128).astype(NP_BF16))
    com["b3t"] = b3.reshape(1, 128).T.copy()
    com["identin"] = np.eye(128, dtype=np.float32)

    in_maps = []
    for c in range(ncores):
        bidx, h = c // 2, c % 2
        sl = slice(h * P, (h + 1) * P)
        X = np.asarray(xyz[bidx], np.float32)[:N]
        X = X - X.mean(0, keepdims=True)
        x2 = (X * X).sum(-1)
        one = np.ones_like(x2)
        djl = np.zeros((128, N), np.float32)
        djl[0:5] = np.stack([X[:, 0], X[:, 1], X[:, 2], x2, one])
        Xi, x2i = X[sl], x2[sl]
        dri = np.zeros((128, P), np.float32)
        dri[0:5] = np.stack([-2 * Xi[:, 0], -2 * Xi[:, 1], -2 * Xi[:, 2],
                             np.ones(P, np.float32), x2i])
        G = np.concatenate(
            [one[:, None], X,
             X[:, [0, 0, 0, 1, 1, 2]] * X[:, [0, 1, 2, 1, 2, 2]]], 1)
        # [N, 10]: 1, x, y, z, xx, xy, xz, yy, yz, zz
        gj = (G.reshape(NB, 128, 10).transpose(1, 0, 2)
              .reshape(128, NB * 10))
        gsgn = np.zeros(10, np.float32)
        for jb in sign_jbs:
            gsgn += G[jb * 128:(jb + 1) * 128].sum(0)
        gfix = G[sl] - 0.5 * gsgn  # [P, 10]
        gfixt = (gfix.reshape(IC, 128, 10).transpose(1, 0, 2)
                 .reshape(128, IC * 10))
        proj = X @ _DIRS.T.astype(np.float32)  # [N, 128]
        pmax = proj.max(0)
        diam = float((pmax[:64] + pmax[64:]).max())
        thr = np.full((128, 1), (FACTOR * diam) ** 2, np.float32)

        m = dict(com)
        h2sl = np.asarray(h2_in[bidx], np.float32)[:, sl]  # [1024, P]
        m["h2s"] = np.ascontiguousarray(
            h2sl.reshape(4, 2, 128, P).transpose(0, 2, 1, 3)
            .reshape(4 * 128, 2 * P)).astype(ml_dtypes.float8_e4m3)
        m["h1s"] = np.ascontiguousarray(
            np.asarray(h1[bidx], np.float32)[:, sl]).astype(NP_BF16)
        m["djl"] = djl.astype(NP_BF16)
        m["dri"] = dri.astype(NP_BF16)
        m["gj"] = gj.astype(NP_BF16)
        m["gfix"] = gfixt.astype(np.float32)
        m["thrin"] = thr
        in_maps.append(m)
    return in_maps


_NC_CACHE = {}


def _get_nc(cfg_key=None):
    if "nc" not in _NC_CACHE:
        _NC_CACHE["nc"] = build_nc(dict(FULL_CFG))
    return _NC_CACHE["nc"]


def kernel(**inputs):
    from concourse.bass_utils import run_bass_kernel_spmd

    xyz = np.asarray(inputs["xyz"], np.float32)
    h1 = np.asarray(inputs["h1"], np.float32)
    h2_in = np.asarray(inputs["h2_in"], np.float32)
    weights = {k: v for k, v in inputs.items()
               if k not in ("xyz", "h1", "h2_in")}

    nc = _get_nc()
    cfg = dict(FULL_CFG)
    in_maps = host_prep(xyz, h1, h2_in, weights, cfg)
    res = run_bass_kernel_spmd(nc, in_maps, core_ids=list(range(NCORES)))
    P = cfg["P"]
    z = np.empty((B, CO, NPTS), np.float32)
    for c in range(NCORES):
        bidx, h = c // 2, c % 2
        z[bidx, :, h * P:(h + 1) * P] = res.results[c]["out"]
    return (inputs["xyz"], z)
